# revision 23
# baseline (speedup 1.0000x reference)
"""Trainium2 Bass kernel for BezierParameterProcessor.

Data-parallel over the batch (character) axis: 1 character per NeuronCore, 8 cores.
All weights are host-prefolded (BN affines, per-scale multipliers, conv im2col
layout) and replicated to every core.

Device pipeline per character:
  1. encoder/agg MLPs (feature-major matmuls)        -> S [256f, 16k]
  2. Bezier points (16 small matmuls) + normalize    -> pn [2, 1600]
  3. ker MLP for all scales -> c_k = 1/(2*softplus^2)
  4. per scale s: att MLP -> softplus(-z); assemble
     rhs5_s = [c*x; c*y; -c; -c*|p|^2; softplus(-z)] (bf16), then the KDE
     grid loop for s: 16-way PE-tiled bf16 matmuls -> (-c*d2 + ln attn) in
     PSUM (split across two chunk tiles A/B), Exp on ACT, row-sums split
     between GPSIMD (halving add) and DVE (reduces).  The att MLP of scale
     s+1 executes in the PE/DVE shadow of scale s's ACT-bound KDE loop.
     After each scale: PE-transpose of the map, scatter into the padded
     channel-major conv input.
  5. conv head: im2col row shifts via contiguous SBUF DMAs, f32r tap
     matmuls, sigmoid+bn3 on a [8,512] layout.
"""

import sys

sys.path.insert(0, "/opt/trn_rl_repo")

import numpy as np
from math import comb
from contextlib import ExitStack

import concourse.bass as bass
import concourse.tile as tile
from concourse import mybir
from concourse.bass_utils import run_bass_kernel_spmd

F32 = mybir.dt.float32
F32R = mybir.dt.float32r
BF16 = mybir.dt.bfloat16
AF = mybir.ActivationFunctionType
ALU = mybir.AluOpType

B, K, R, D = 8, 16, 100, 256
N = K * R            # 1600
H = W = 64
G = H * W            # 4096
NCORES = 8
BN_EPS = 1e-5
NTILES_A = [(0, 512), (512, 512)]          # psum chunk A: n-cols 0..1023
NTILES_B = [(1024, 512), (1536, 64)]       # psum chunk B: n-cols 1024..1599
NB = 576                                   # chunk B width


def _r(ap):
    return ap.bitcast(F32R)


def _host_constants():
    t = np.linspace(0.0, 1.0, R).astype(np.float64)
    basisT = np.stack(
        [comb(3, c) * t**c * (1.0 - t) ** (3 - c) for c in range(4)], axis=0
    ).astype(np.float32)                               # [4, 100]

    onehot = np.zeros((K, N), np.float32)
    for k in range(K):
        onehot[k, k * R : (k + 1) * R] = 1.0           # [16, 1600]

    xs = np.linspace(0.0, 1.0, W).astype(np.float64)
    gx = np.tile(xs, H)                                 # g % 64
    gy = np.repeat(xs, W)                               # g // 64
    grid5 = np.stack(
        [2.0 * gx, 2.0 * gy, gx**2 + gy**2, np.ones(G), -np.ones(G)], axis=0
    ).astype(np.float32)                                # [5, 4096]
    # replicated at partition offsets 0/32/64/96 for 16-way PE tile packing
    gridT = np.zeros((128, G), np.float32)
    for i in range(4):
        gridT[32 * i : 32 * i + 5, :] = grid5
    # sign mask for building rhs rows, replicated at the same offsets
    mask101 = np.zeros((16, 101), np.float32)
    pat = np.array([1.0, 1.0, -1.0, -1.0, 0.0], np.float32)
    for i in range(4):
        mask101[:, 32 * i : 32 * i + 5] = pat[None, :]
    id128 = np.eye(128, dtype=np.float32)
    return basisT, onehot, gridT, mask101, id128


def _split_multi_waits(nc):
    """Walrus codegen in this toolchain accepts one sync-wait per instruction;
    carry extra waits on same-engine NoOps inserted just before."""
    for f in nc.m.functions:
        for blk in f.blocks:
            idx = 0
            while idx < len(blk.instructions):
                inst = blk.instructions[idx]
                si = inst.sync_info
                if si is not None and len(si.on_wait) > 1:
                    waits = list(si.on_wait)
                    for j, w in enumerate(waits[:-1]):
                        nop = mybir.InstNoOp(name=f"WSPLIT-{nc.next_id()}",
                                             ins=[], outs=[])
                        nop.engine = inst.engine
                        nop.sync_info = mybir.SyncInfo(on_wait=[w], on_update=[])
                        blk.instructions.insert(idx + j, nop)
                    idx += len(waits) - 1
                    inst.sync_info = mybir.SyncInfo(on_wait=[waits[-1]],
                                                    on_update=list(si.on_update))
                idx += 1


def _build_program():
    nc = bass.Bass()

    # ---- DRAM I/O declarations (shapes only; data supplied per core) ----
    dr = {}

    def din(name, shape, dt=F32):
        dr[name] = nc.dram_tensor(name, list(shape), dt, kind="ExternalInput")
        return dr[name]

    din("cpT", (2, 64))            # encoder input, cols n=(k,cp)
    din("cpd", (4, 32))            # bezier lhsT, cols 2k+d
    din("basisT", (4, 100))
    din("onehot", (16, N), F32R)
    din("gridT", (128, G), BF16)
    din("mask101", (16, 101))
    din("id128", (128, 128))
    din("encw1", (2, 64)), din("encb1", (64, 1))
    din("encw2", (64, 128)), din("encb2", (128, 1))
    din("encw3", (128, 256)), din("encb3", (128, 2))
    din("aggw1", (128, 2, 2, 128)), din("aggb1", (128, 2))
    din("aggw2", (128, 2, 2, 128)), din("aggb2", (128, 2))
    din("kerw1", (128, 2, 3, 64), F32R), din("kerb1", (64, 3))
    din("kerw2", (64, 32)), din("kerb2", (32, 1))
    din("kerw3", (32, 1))
    din("aw1", (128, 2, 3, 256), F32R), din("ab1row", (1, 3, 256), F32R)
    din("w1p", (2, 256), F32R)
    din("attw2", (128, 2, 128), BF16), din("attb2", (128, 1))
    din("attw3", (128, 1), F32R)
    din("w1im", (128, 3, 16), F32R), din("fusb1", (16, 1))
    din("w2im", (128, 3, 8), F32R), din("fusb2", (8, 1))
    din("w3sel", (8, 8, 8), F32R)
    out_dram = nc.dram_tensor("out", [1, G], F32, kind="ExternalOutput")

    # imm scalars get baked at build time from the actual inputs:
    # we return a closure that finishes the build given those values.
    def finish(attb3, kerb3, fusb3, bn3f, bn3b):
        with ExitStack() as ctx:
            tc = ctx.enter_context(tile.TileContext(nc))
            cpool = ctx.enter_context(tc.tile_pool(name="consts", bufs=1))
            wpool = ctx.enter_context(tc.tile_pool(name="work", bufs=1))

            # ---- load constants/weights to SBUF (big late-use ones last) ----
            sb = {}
            for name, shape in [
                ("cpT", (2, 64)), ("cpd", (4, 32)), ("basisT", (4, 100)),
                ("mask101", (16, 101)),
                ("encw1", (2, 64)), ("encb1", (64, 1)),
                ("encw2", (64, 128)), ("encb2", (128, 1)),
                ("encw3", (128, 256)), ("encb3", (128, 2)),
                ("aggw1", (128, 2, 2, 128)), ("aggb1", (128, 2)),
                ("aggw2", (128, 2, 2, 128)), ("aggb2", (128, 2)),
                ("kerw1", (128, 2, 3, 64)), ("kerb1", (64, 3)),
                ("kerw2", (64, 32)), ("kerb2", (32, 1)),
                ("kerw3", (32, 1)),
                ("aw1", (128, 2, 3, 256)), ("ab1row", (1, 3, 256)),
                ("attw2", (128, 2, 128)), ("attb2", (128, 1)), ("attw3", (128, 1)),
                ("w1im", (128, 3, 16)), ("fusb1", (16, 1)),
                ("w2im", (128, 3, 8)), ("fusb2", (8, 1)),
                ("w3sel", (8, 8, 8)),
                ("onehot", (16, N)), ("id128", (128, 128)),
                ("gridT", (128, G)),
            ]:
                sb[name] = cpool.tile(list(shape), dr[name].dtype,
                                      name=f"sb_{name}")
                nc.sync.dma_start(out=sb[name][...], in_=dr[name][...])

            ones2 = cpool.tile([2, 1], F32R)
            nc.vector.memset(ones2[...].bitcast(F32), 1.0)
            ones16 = cpool.tile([1, 16], F32R)
            nc.vector.memset(ones16[...].bitcast(F32), 1.0)
            kerb3_t = cpool.tile([1, 1], F32)
            nc.vector.memset(kerb3_t[...], float(kerb3))
            nattb3_t = cpool.tile([1, 1], F32)
            nc.vector.memset(nattb3_t[...], float(-attb3))
            fusb3_t = cpool.tile([8, 1], F32)
            nc.vector.memset(fusb3_t[...], float(fusb3))

            # conv buffers allocated early; border memsets overlap early phases
            cvsb = ctx.enter_context(tc.tile_pool(name="conv_sbuf", bufs=1))
            # disjoint lifetimes share a slot: mpad dies once imY is built
            # (c2u reuses it)
            mpad = cvsb.tile([3, 66, 66], F32R, tag="cshare1")
            mTs = cvsb.tile([16, 2, 3, 128], F32R)
            c1p = cvsb.tile([16, 66, 66], F32R)
            imY = cvsb.tile([128, 64, 66], F32R)
            imY2 = cvsb.tile([128, 64, 66], F32R)
            c2u = cvsb.tile([8, 64, 64], F32R, tag="cshare1")
            sg = cvsb.tile([8, 512], F32)
            for t in (mpad, c1p):
                nc.vector.memset(t[:, 0:1, :].bitcast(F32), 0.0)     # top row
                nc.vector.memset(t[:, 65:66, :].bitcast(F32), 0.0)   # bottom row
                nc.vector.memset(t[:, 1:65, 0:1].bitcast(F32), 0.0)  # left col
                nc.vector.memset(t[:, 1:65, 65:66].bitcast(F32), 0.0)  # right col

            # ============ Phase 1: encoder + agg (feature-major) ============
            h1 = wpool.tile([64, 64], F32)
            h2 = wpool.tile([128, 64], F32)
            h3 = wpool.tile([128, 2, 64], F32)
            m = wpool.tile([128, 2, 16], F32)
            g1 = wpool.tile([128, 2, 16], F32)
            S = wpool.tile([128, 2, 16], F32R)

            with tc.tile_pool(name="pp1", bufs=4, space="PSUM") as pp1:
                ps = pp1.tile([64, 64], F32, tag="pp1t")
                nc.tensor.matmul(ps[...], sb["encw1"][...], sb["cpT"][...],
                                 start=True, stop=True)
                nc.scalar.activation(h1[...], ps[...], AF.Relu, bias=sb["encb1"][:, 0:1])

                ps2 = pp1.tile([128, 64], F32, tag="pp1t")
                nc.tensor.matmul(ps2[...], sb["encw2"][...], h1[...],
                                 start=True, stop=True)
                nc.scalar.activation(h2[...], ps2[...], AF.Relu, bias=sb["encb2"][:, 0:1])

                for fh in range(2):
                    ps3 = pp1.tile([128, 64], F32, tag="pp1t")
                    nc.tensor.matmul(ps3[...], sb["encw3"][:, 128 * fh : 128 * (fh + 1)],
                                     h2[...], start=True, stop=True)
                    nc.scalar.activation(h3[:, fh, :], ps3[...], AF.Relu,
                                         bias=sb["encb3"][:, fh : fh + 1])

                # mean over 4 control points (the 0.25 is folded into aggw1)
                h3r = h3[...].rearrange("p h (k c) -> p h k c", c=4)
                nc.vector.tensor_add(m[...], h3r[:, :, :, 0], h3r[:, :, :, 1])
                nc.vector.tensor_add(m[...], m[...], h3r[:, :, :, 2])
                nc.vector.tensor_add(m[...], m[...], h3r[:, :, :, 3])

                for dst, wname, bname, rhs in ((g1, "aggw1", "aggb1", m),
                                               (S, "aggw2", "aggb2", g1)):
                    for fh in range(2):
                        psg = pp1.tile([128, 16], F32, tag="pp1t")
                        for inh in range(2):
                            nc.tensor.matmul(psg[...], sb[wname][:, inh, fh, :],
                                             rhs[:, inh, :],
                                             start=(inh == 0), stop=(inh == 1))
                        nc.scalar.activation(dst[:, fh, :], psg[...], AF.Relu,
                                             bias=sb[bname][:, fh : fh + 1])

                # ============ Phase 2: Bezier points ============
                P = wpool.tile([2, N], F32)
                for k in range(K):
                    psb = pp1.tile([2, 100], F32, tag="pp1t")
                    nc.tensor.matmul(psb[...], sb["cpd"][:, 2 * k : 2 * k + 2],
                                     sb["basisT"][...], start=True, stop=True)
                    nc.vector.tensor_copy(P[:, R * k : R * (k + 1)], psb[...])

                pmin = wpool.tile([2, 1], F32)
                pmax = wpool.tile([2, 1], F32)
                rec = wpool.tile([2, 1], F32)
                nc.vector.tensor_reduce(pmin[...], P[...], axis=mybir.AxisListType.X,
                                        op=ALU.min)
                nc.vector.tensor_reduce(pmax[...], P[...], axis=mybir.AxisListType.X,
                                        op=ALU.max)
                nc.vector.tensor_tensor(rec[...], pmax[...], pmin[...], op=ALU.subtract)
                nc.vector.tensor_scalar_add(rec[...], rec[...], 1e-8)
                nc.vector.reciprocal(rec[...], rec[...])
                # pn = (P - pmin) * rec, in place
                nc.vector.tensor_scalar(P[...], P[...], pmin[...], rec[...],
                                        op0=ALU.subtract, op1=ALU.mult)

                P2 = wpool.tile([2, N], F32R)
                nc.vector.tensor_mul(P2[...], P[...], P[...])
                sqrow = wpool.tile([1, N], F32)
                for t0, w in NTILES_A + NTILES_B:
                    pss = pp1.tile([1, 512], F32, tag="pp1t")
                    nc.tensor.matmul(pss[:, :w], ones2[...], P2[:, t0 : t0 + w],
                                     start=True, stop=True)
                    nc.vector.tensor_copy(sqrow[:, t0 : t0 + w], pss[:, :w])

                # ---- ker MLP for all 3 scales -> cT [16, 3] ----
                e48 = wpool.tile([1, 48], F32)
                r48 = wpool.tile([1, 48], F32)
                cT = wpool.tile([16, 3], F32)
                k1 = wpool.tile([64, 16], F32)
                k2 = wpool.tile([32, 16], F32)
                for s in range(3):
                    psk1 = pp1.tile([64, 16], F32, tag="pp1t")
                    for inh in range(2):
                        nc.tensor.matmul(psk1[...], sb["kerw1"][:, inh, s, :],
                                         S[:, inh, :], start=(inh == 0), stop=(inh == 1))
                    nc.scalar.activation(k1[...], psk1[...], AF.Relu,
                                         bias=sb["kerb1"][:, s : s + 1])
                    psk2 = pp1.tile([32, 16], F32, tag="pp1t")
                    nc.tensor.matmul(psk2[...], sb["kerw2"][...], k1[...],
                                     start=True, stop=True)
                    nc.scalar.activation(k2[...], psk2[...], AF.Relu,
                                         bias=sb["kerb2"][:, 0:1])
                    psk3 = pp1.tile([1, 16], F32, tag="pp1t")
                    nc.tensor.matmul(psk3[...], sb["kerw3"][...], k2[...],
                                     start=True, stop=True)
                    # e48 k-major (col 3k+s) = exp(z + kerb3)
                    e48v = e48[...].rearrange("p (k s) -> p k s", s=3)
                    nc.scalar.activation(e48v[:, :, s], psk3[...],
                                         AF.Exp, bias=kerb3_t[...])

                # softplus, then c = 1/(2*sp^2) = (recip(sp)/sqrt(2))^2
                nc.vector.tensor_scalar_add(e48[...], e48[...], 1.0)
                nc.scalar.activation(e48[...], e48[...], AF.Ln)
                nc.vector.reciprocal(r48[...], e48[...])
                nc.scalar.activation(e48[...], r48[...], AF.Square,
                                     scale=0.7071067811865476)
                # transpose [1,48] -> [16k, 3s]
                nc.sync.dma_start(out=cT[...], in_=e48[...])

            # assembled point-side tensors
            xaug = wpool.tile([18, N], F32R)
            nc.sync.dma_start(out=xaug[0:16, :], in_=sb["onehot"][...])
            nc.sync.dma_start(out=xaug[16:18, :], in_=_r(P[...]))
            prep101 = wpool.tile([101, N], F32)
            nc.vector.memset(prep101[...], 1.0)
            for i in range(4):
                nc.sync.dma_start(out=prep101[32 * i : 32 * i + 2, :], in_=P[...])
                nc.sync.dma_start(out=prep101[32 * i + 3 : 32 * i + 4, :],
                                  in_=sqrow[...])

            # w1aug rows 16,17 = w1p for every scale (broadcast DMA)
            w1aug = wpool.tile([18, 3, 256], F32R)
            for s in range(3):
                nc.sync.dma_start(out=w1aug[16:18, s, :], in_=dr["w1p"][...])

            # ====== Phases 3+4 interleaved: per-scale att MLP + KDE ======
            rhs5 = [
                wpool.tile([101, N], BF16, name=f"rhs5_{s}", tag=f"rhs5_{s}")
                for s in range(3)
            ]
            m_all = wpool.tile([128, 3, 32], F32)

            with (
                tc.tile_pool(name="scale_work", bufs=1) as spool,
                tc.tile_pool(name="mlp_ps", bufs=4, space="PSUM") as pp3,
            ):
                # the 3 scales' chains are independent: emit stage-major so
                # the PE streams while DVE/ACT chase, no serial per-scale wall
                a1 = [spool.tile([128, 2, N], BF16, name=f"a1_{s}", tag=f"a1_{s}")
                      for s in range(3)]
                a2 = [spool.tile([128, N], F32R, name=f"a2_{s}", tag=f"a2_{s}")
                      for s in range(3)]
                esp = [spool.tile([1, N], BF16, name=f"esp_{s}", tag=f"esp_{s}")
                       for s in range(3)]
                cneg5 = [spool.tile([16, 101], F32R, name=f"cneg5_{s}",
                         tag=f"cneg5_{s}") for s in range(3)]
                for s in range(3):
                    pscf = pp3.tile([16, 256], F32, tag="mlp")
                    nc.tensor.matmul(pscf[...], S[:, 0, :], sb["aw1"][:, 0, s, :],
                                     start=True, stop=False)
                    nc.tensor.matmul(pscf[...], S[:, 1, :], sb["aw1"][:, 1, s, :],
                                     start=False, stop=False)
                    nc.tensor.matmul(pscf[...], ones16[...],
                                     sb["ab1row"][:, s, :], start=False, stop=True)
                    nc.vector.tensor_copy(_r(w1aug[0:16, s, :]), pscf[...])
                    nc.vector.tensor_scalar_mul(cneg5[s][...], _r(sb["mask101"][...]),
                                                cT[:, s : s + 1])
                for fh in range(2):
                    for t0, w in NTILES_A + NTILES_B:
                        for s in range(3):
                            psa = pp3.tile([128, 512], F32, tag="mlp")
                            nc.tensor.matmul(psa[:, :w],
                                             w1aug[:, s, 128 * fh : 128 * (fh + 1)],
                                             xaug[:, t0 : t0 + w],
                                             start=True, stop=True)
                            nc.vector.tensor_scalar_max(a1[s][:, fh, t0 : t0 + w],
                                                        psa[:, :w], 0.0)
                for t0, w in NTILES_A + NTILES_B:
                    for s in range(3):
                        psa2 = pp3.tile([128, 512], F32, tag="mlp")
                        for fh in range(2):
                            nc.tensor.matmul(psa2[:, :w], sb["attw2"][:, fh, :],
                                             a1[s][:, fh, t0 : t0 + w],
                                             start=(fh == 0), stop=(fh == 1))
                        nc.vector.tensor_scalar(a2[s][:, t0 : t0 + w], psa2[:, :w],
                                                sb["attb2"][:, 0:1], 0.0,
                                                op0=ALU.add, op1=ALU.max)
                for t0, w in NTILES_A + NTILES_B:
                    for s in range(3):
                        psz = pp3.tile([1, 512], F32, tag="mlp")
                        nc.tensor.matmul(psz[:, :w], sb["attw3"][...],
                                         a2[s][:, t0 : t0 + w], start=True, stop=True)
                        # exp(-(z + attb3))
                        nc.scalar.activation(esp[s][:, t0 : t0 + w], psz[:, :w],
                                             AF.Exp, bias=nattb3_t[...], scale=-1.0)
                for s in range(3):
                    nc.vector.tensor_scalar_add(esp[s][...], esp[s][...], 1.0)
                    nc.scalar.activation(esp[s][...], esp[s][...], AF.Ln)
                for t0, w in NTILES_A + NTILES_B:
                    for s in range(3):
                        psc = pp3.tile([101, 512], F32, tag="mlp")
                        nc.tensor.matmul(psc[:, :w], cneg5[s][...],
                                         sb["onehot"][:, t0 : t0 + w],
                                         start=True, stop=True)
                        nc.vector.tensor_mul(rhs5[s][:, t0 : t0 + w], psc[:, :w],
                                             prep101[:, t0 : t0 + w])
                # rows 32i+4 <- softplus(-z-b3)  (partition move via DMA)
                for s in range(3):
                    for i in range(4):
                        nc.sync.dma_start(out=rhs5[s][32 * i + 4 : 32 * i + 5, :],
                                          in_=esp[s][...])

            # ====== Phase 4: KDE (single exp per block, ACT accumulator) ====
            with (
                tc.tile_pool(name="kde_ps", bufs=2, space="PSUM") as kpp,
                tc.tile_pool(name="kde_scratch", bufs=2) as ksp,
            ):
                for s in range(3):
                    for gb in range(32):
                        kps = kpp.tile([128, 2048], F32, tag="kps")
                        for ti, (t0, w) in enumerate(NTILES_A + NTILES_B):
                            for j in range(4):
                                nc.tensor.matmul(
                                    kps[32 * j : 32 * (j + 1),
                                        512 * ti : 512 * ti + w],
                                    sb["gridT"][32 * ti : 32 * ti + 5,
                                               128 * gb + 32 * j : 128 * gb + 32 * (j + 1)],
                                    rhs5[s][32 * ti : 32 * ti + 5, t0 : t0 + w],
                                    start=True, stop=True,
                                    tile_position=(32 * ti, 32 * j),
                                )
                        scr = ksp.tile([128, N], BF16, tag="scr")
                        nc.scalar.activation(scr[...], kps[:, 0:N], AF.Exp,
                                             accum_out=m_all[:, s, gb : gb + 1])

                        # after each half of the grid, transpose the
                        # finished 16 gb-columns and scatter them so only the
                        # final half of scale 2 sits on the critical path.
                        if gb in (15, 31):
                            g0 = gb - 15
                            pst = kpp.tile([16, 128], F32, tag="kps", bufs=2)
                            nc.tensor.transpose(pst[...],
                                                m_all[:, s, g0 : g0 + 16],
                                                sb["id128"][...])
                            hb = g0 // 16
                            nc.vector.tensor_copy(mTs[:, hb, s, :], pst[...])
                            nc.sync.dma_start(
                                out=mpad[s : s + 1, 1 + 2 * g0 : 33 + 2 * g0,
                                         1:65].rearrange(
                                    "a (gb ph) x -> a gb ph x", ph=2),
                                in_=mTs[:, hb, s, :],
                            )
                            # im2col rows for this channel, replicated into
                            # all four PE row groups:
                            # imY[32q+3dy+s, y, x] = mpad[s, y+dy, x]
                            y0, y1 = (0, 30) if g0 == 0 else (30, 64)
                            for q in range(4):
                                for dy in range(3):
                                    nc.sync.dma_start(
                                        out=imY[32 * q + 3 * dy + s
                                                : 32 * q + 3 * dy + s + 1,
                                                y0:y1, :],
                                        in_=mpad[s : s + 1, y0 + dy : y1 + dy, :])

            # ============ Phase 5: conv head ============
            with tc.tile_pool(name="conv_ps", bufs=4, space="PSUM") as cvp:
                for st in range(8):
                    q = st % 4
                    ps1 = cvp.tile([16, 512], F32, tag="cv1")
                    for dx in range(3):
                        nc.tensor.matmul(
                            ps1[...], sb["w1im"][32 * q : 32 * q + 9, dx, :],
                            imY[32 * q : 32 * q + 9,
                                st * 8 : st * 8 + 8, dx : dx + 64],
                            start=(dx == 0), stop=(dx == 2),
                            tile_position=(32 * q, 0),
                        )
                    nc.vector.tensor_scalar(c1p[:, 1 + st * 8 : 9 + st * 8, 1:65],
                                            ps1[...], sb["fusb1"][:, 0:1], 0.0,
                                            op0=ALU.add, op1=ALU.max)
                # imY2[64q2+16dy+c, y, x] = c1p[c, y+dy, x]; per-strip chunks
                # so conv2 strips can start while conv1 is still running
                for st in range(8):
                    for q2 in range(2):
                        for dy in range(3):
                            nc.sync.dma_start(
                                out=imY2[64 * q2 + 16 * dy
                                         : 64 * q2 + 16 * dy + 16,
                                         st * 8 : st * 8 + 8, :],
                                in_=c1p[:, st * 8 + dy : st * 8 + 8 + dy, :])
                for st in range(8):
                    q2 = st % 2
                    ps2c = cvp.tile([8, 512], F32, tag="cv2")
                    for dx in range(3):
                        nc.tensor.matmul(
                            ps2c[...], sb["w2im"][64 * q2 : 64 * q2 + 48, dx, :],
                            imY2[64 * q2 : 64 * q2 + 48,
                                 st * 8 : st * 8 + 8, dx : dx + 64],
                            start=(dx == 0), stop=(dx == 2),
                            tile_position=(64 * q2, 0),
                        )
                    nc.vector.tensor_scalar(c2u[:, st * 8 : 8 + st * 8, :],
                                            ps2c[...], sb["fusb2"][:, 0:1], 0.0,
                                            op0=ALU.add, op1=ALU.max)
            with tc.tile_pool(name="conv3_ps", bufs=1, space="PSUM") as cvp3:
                # 1x1 conv with strip-selector weights: psum row st = w3 . c2u strip st
                ps3c = cvp3.tile([8, 512], F32, tag="cv3")
                for st in range(8):
                    nc.tensor.matmul(ps3c[...],
                                     sb["w3sel"][:, st, :],
                                     c2u[:, st * 8 : st * 8 + 8, :],
                                     start=(st == 0), stop=(st == 7))
                nc.scalar.activation(sg[...], ps3c[...], AF.Sigmoid,
                                     bias=fusb3_t[...])

            # bn3 affine, then store
            nc.vector.tensor_scalar(sg[...], sg[...], bn3f, bn3b,
                                    op0=ALU.mult, op1=ALU.add)
            nc.sync.dma_start(
                out=out_dram[...].rearrange("a (r x) -> (a r) x", r=8),
                in_=sg[...])

        _split_multi_waits(nc)
        return nc

    return nc, finish


def _prepare_maps(inputs):
    """Host-side weight folding; returns per-core in_maps (list of dicts)."""
    f = {k: np.asarray(v, dtype=np.float32) for k, v in inputs.items()}
    basisT, onehot, gridT, mask101, id128 = _host_constants()

    bn1f = f["bn1_g"] / np.sqrt(np.float32(1.0 + BN_EPS))
    bn2f = f["bn2_g"] / np.sqrt(np.float32(1.0 + BN_EPS))
    A = (bn1f * bn2f).astype(np.float32)                     # [256]
    C = (f["bn1_b"] * bn2f + f["bn2_b"]).astype(np.float32)  # [256]

    scales = (0.5, 1.0, 2.0)
    kerw1 = np.stack(
        [(s * A)[:, None] * f["ker_w1"] for s in scales], 0
    )  # [3,256,64]
    kerb1 = np.stack(
        [s * (C @ f["ker_w1"]) + f["ker_b1"] for s in scales], 1
    )  # [64,3]
    aw1f = np.stack(
        [(s * A)[:, None] * f["att_w1"][:D] for s in scales], 0
    )  # [3,256,256]
    ab1row = np.stack(
        [s * (C @ f["att_w1"][:D]) + f["att_b1"] for s in scales], 0
    ).reshape(1, 3, 256)

    w1im9 = f["fus_w1"].transpose(2, 1, 3, 0).reshape(9, 3, 16)   # [3dy+c, dx, o]
    w1im = np.zeros((128, 3, 16), np.float32)
    for q in range(4):
        w1im[32 * q : 32 * q + 9] = w1im9                          # 4 row-group copies
    w2im48 = f["fus_w2"].transpose(2, 1, 3, 0).reshape(48, 3, 8)   # [16dy+c, dx, o]
    w2im = np.zeros((128, 3, 8), np.float32)
    for q in range(2):
        w2im[64 * q : 64 * q + 48] = w2im48                        # 2 row-group copies
    w3 = f["fus_w3"].reshape(8)
    w3sel = np.zeros((8, 8, 8), np.float32)                       # [c, st, r]
    for st in range(8):
        w3sel[:, st, st] = w3

    import ml_dtypes
    common = {
        "basisT": basisT,
        "onehot": onehot,
        "gridT": gridT.astype(ml_dtypes.bfloat16),
        "mask101": mask101,
        "id128": id128,
        "encw1": f["enc_w1"],
        "encb1": f["enc_b1"].reshape(64, 1),
        "encw2": f["enc_w2"],
        "encb2": f["enc_b2"].reshape(128, 1),
        "encw3": f["enc_w3"],
        "encb3": f["enc_b3"].reshape(2, 128).T.copy(),
        "aggw1": (0.25 * f["agg_w1"]).reshape(2, 128, 2, 128).transpose(1, 0, 2, 3).copy(),
        "aggb1": f["agg_b1"].reshape(2, 128).T.copy(),
        "aggw2": f["agg_w2"].reshape(2, 128, 2, 128).transpose(1, 0, 2, 3).copy(),
        "aggb2": f["agg_b2"].reshape(2, 128).T.copy(),
        "kerw1": kerw1.reshape(3, 2, 128, 64).transpose(2, 1, 0, 3).copy(),
        "kerb1": kerb1,
        "kerw2": f["ker_w2"],
        "kerb2": f["ker_b2"].reshape(32, 1),
        "kerw3": f["ker_w3"],
        "aw1": aw1f.reshape(3, 2, 128, 256).transpose(2, 1, 0, 3).copy(),
        "ab1row": ab1row,
        "w1p": f["att_w1"][D : D + 2].copy(),
        "attw2": f["att_w2"].reshape(2, 128, 128).transpose(1, 0, 2).astype(ml_dtypes.bfloat16),
        "attb2": f["att_b2"].reshape(128, 1),
        "attw3": f["att_w3"],
        "w1im": w1im,
        "fusb1": f["fus_b1"].reshape(16, 1),
        "w2im": w2im,
        "fusb2": f["fus_b2"].reshape(8, 1),
        "w3sel": w3sel,
    }
    common = {
        k: np.ascontiguousarray(v) if v.dtype == ml_dtypes.bfloat16
        else np.ascontiguousarray(v, dtype=np.float32)
        for k, v in common.items()
    }

    in_maps = []
    cp = f["control_points"]  # [8, 16, 4, 2]
    for c in range(NCORES):
        m = dict(common)
        m["cpT"] = np.ascontiguousarray(cp[c].reshape(64, 2).T)       # [2, 64]
        m["cpd"] = np.ascontiguousarray(cp[c].transpose(1, 0, 2).reshape(4, 32))
        in_maps.append(m)

    imm = dict(
        attb3=float(f["att_b3"][0]),
        kerb3=float(f["ker_b3"][0]),
        fusb3=float(f["fus_b3"][0]),
        bn3f=float(f["bn3_g"][0] / np.sqrt(1.0 + BN_EPS)),
        bn3b=float(f["bn3_b"][0]),
    )
    return in_maps, imm


def kernel(**inputs) -> np.ndarray:
    in_maps, imm = _prepare_maps(inputs)
    nc, finish = _build_program()
    nc = finish(**imm)
    res = run_bass_kernel_spmd(nc, in_maps, core_ids=list(range(NCORES)))
    kernel._last_results = res
    out = np.stack([r["out"].reshape(1, H, W) for r in res.results], axis=0)
    return out.astype(np.float32)


# revision 24
# speedup vs baseline: 1.0447x; 1.0447x over previous
"""Trainium2 Bass kernel for BezierParameterProcessor.

Data-parallel over the batch (character) axis: 1 character per NeuronCore, 8 cores.
All weights are host-prefolded (BN affines, per-scale multipliers, conv im2col
layout) and replicated to every core.

Device pipeline per character:
  1. encoder/agg MLPs (feature-major matmuls)        -> S [256f, 16k]
  2. Bezier points (16 small matmuls) + normalize    -> pn [2, 1600]
  3. ker MLP for all scales -> c_k = 1/(2*softplus^2)
  4. per scale s: att MLP -> softplus(-z); assemble
     rhs5_s = [c*x; c*y; -c; -c*|p|^2; softplus(-z)] (bf16), then the KDE
     grid loop for s: 16-way PE-tiled bf16 matmuls -> (-c*d2 + ln attn) in
     PSUM (split across two chunk tiles A/B), Exp on ACT, row-sums split
     between GPSIMD (halving add) and DVE (reduces).  The att MLP of scale
     s+1 executes in the PE/DVE shadow of scale s's ACT-bound KDE loop.
     After each scale: PE-transpose of the map, scatter into the padded
     channel-major conv input.
  5. conv head: im2col row shifts via contiguous SBUF DMAs, f32r tap
     matmuls, sigmoid+bn3 on a [8,512] layout.
"""

import sys

sys.path.insert(0, "/opt/trn_rl_repo")

import numpy as np
from math import comb
from contextlib import ExitStack

import concourse.bass as bass
import concourse.tile as tile
from concourse import mybir
from concourse.bass_utils import run_bass_kernel_spmd

F32 = mybir.dt.float32
F32R = mybir.dt.float32r
BF16 = mybir.dt.bfloat16
AF = mybir.ActivationFunctionType
ALU = mybir.AluOpType

B, K, R, D = 8, 16, 100, 256
N = K * R            # 1600
H = W = 64
G = H * W            # 4096
NCORES = 8
BN_EPS = 1e-5
NTILES_A = [(0, 512), (512, 512)]          # psum chunk A: n-cols 0..1023
NTILES_B = [(1024, 512), (1536, 64)]       # psum chunk B: n-cols 1024..1599
NB = 576                                   # chunk B width


def _r(ap):
    return ap.bitcast(F32R)


def _host_constants():
    t = np.linspace(0.0, 1.0, R).astype(np.float64)
    basisT = np.stack(
        [comb(3, c) * t**c * (1.0 - t) ** (3 - c) for c in range(4)], axis=0
    ).astype(np.float32)                               # [4, 100]

    onehot = np.zeros((K, N), np.float32)
    for k in range(K):
        onehot[k, k * R : (k + 1) * R] = 1.0           # [16, 1600]

    xs = np.linspace(0.0, 1.0, W).astype(np.float64)
    gx = np.tile(xs, H)                                 # g % 64
    gy = np.repeat(xs, W)                               # g // 64
    grid5 = np.stack(
        [2.0 * gx, 2.0 * gy, gx**2 + gy**2, np.ones(G), -np.ones(G)], axis=0
    ).astype(np.float32)                                # [5, 4096]
    # replicated at partition offsets 0/32/64/96 for 16-way PE tile packing
    gridT = np.zeros((128, G), np.float32)
    for i in range(4):
        gridT[32 * i : 32 * i + 5, :] = grid5
    # sign mask for building rhs rows, replicated at the same offsets
    mask101 = np.zeros((16, 101), np.float32)
    pat = np.array([1.0, 1.0, -1.0, -1.0, 0.0], np.float32)
    for i in range(4):
        mask101[:, 32 * i : 32 * i + 5] = pat[None, :]
    id128 = np.eye(128, dtype=np.float32)
    return basisT, onehot, gridT, mask101, id128


def _split_multi_waits(nc):
    """Walrus codegen in this toolchain accepts one sync-wait per instruction;
    carry extra waits on same-engine NoOps inserted just before."""
    for f in nc.m.functions:
        for blk in f.blocks:
            idx = 0
            while idx < len(blk.instructions):
                inst = blk.instructions[idx]
                si = inst.sync_info
                if si is not None and len(si.on_wait) > 1:
                    waits = list(si.on_wait)
                    for j, w in enumerate(waits[:-1]):
                        nop = mybir.InstNoOp(name=f"WSPLIT-{nc.next_id()}",
                                             ins=[], outs=[])
                        nop.engine = inst.engine
                        nop.sync_info = mybir.SyncInfo(on_wait=[w], on_update=[])
                        blk.instructions.insert(idx + j, nop)
                    idx += len(waits) - 1
                    inst.sync_info = mybir.SyncInfo(on_wait=[waits[-1]],
                                                    on_update=list(si.on_update))
                idx += 1


def _build_program():
    nc = bass.Bass()

    # ---- DRAM I/O declarations (shapes only; data supplied per core) ----
    dr = {}

    def din(name, shape, dt=F32):
        dr[name] = nc.dram_tensor(name, list(shape), dt, kind="ExternalInput")
        return dr[name]

    din("cpT", (2, 64))            # encoder input, cols n=(k,cp)
    din("cpd", (4, 32))            # bezier lhsT, cols 2k+d
    din("basisT", (4, 100))
    din("onehot", (16, N), F32R)
    din("gridT", (128, G), BF16)
    din("mask101", (16, 101))
    din("id128", (128, 128))
    din("encw1", (2, 64)), din("encb1", (64, 1))
    din("encw2", (64, 128)), din("encb2", (128, 1))
    din("encw3", (128, 256)), din("encb3", (128, 2))
    din("aggw1", (128, 2, 2, 128)), din("aggb1", (128, 2))
    din("aggw2", (128, 2, 2, 128)), din("aggb2", (128, 2))
    din("kerw1", (128, 2, 3, 64), F32R), din("kerb1", (64, 3))
    din("kerw2", (64, 32)), din("kerb2", (32, 1))
    din("kerw3", (32, 1))
    din("aw1", (128, 2, 3, 256), F32R), din("ab1row", (1, 3, 256), F32R)
    din("w1p", (2, 256), F32R)
    din("attw2", (128, 2, 128), BF16), din("attb2", (128, 1))
    din("attw3", (128, 1), F32R)
    din("w1im", (128, 3, 16), F32R), din("fusb1", (16, 1))
    din("w2im", (128, 3, 8), F32R), din("fusb2", (8, 1))
    din("w3sel", (8, 8, 8), F32R)
    out_dram = nc.dram_tensor("out", [1, G], F32, kind="ExternalOutput")

    # imm scalars get baked at build time from the actual inputs:
    # we return a closure that finishes the build given those values.
    def finish(attb3, kerb3, fusb3, bn3f, bn3b):
        with ExitStack() as ctx:
            tc = ctx.enter_context(tile.TileContext(nc))
            cpool = ctx.enter_context(tc.tile_pool(name="consts", bufs=1))
            wpool = ctx.enter_context(tc.tile_pool(name="work", bufs=1))

            # ---- load constants/weights to SBUF (big late-use ones last) ----
            sb = {}
            for name, shape in [
                ("cpT", (2, 64)), ("cpd", (4, 32)), ("basisT", (4, 100)),
                ("mask101", (16, 101)),
                ("encw1", (2, 64)), ("encb1", (64, 1)),
                ("encw2", (64, 128)), ("encb2", (128, 1)),
                ("encw3", (128, 256)), ("encb3", (128, 2)),
                ("aggw1", (128, 2, 2, 128)), ("aggb1", (128, 2)),
                ("aggw2", (128, 2, 2, 128)), ("aggb2", (128, 2)),
                ("kerw1", (128, 2, 3, 64)), ("kerb1", (64, 3)),
                ("kerw2", (64, 32)), ("kerb2", (32, 1)),
                ("kerw3", (32, 1)),
                ("aw1", (128, 2, 3, 256)), ("ab1row", (1, 3, 256)),
                ("attw2", (128, 2, 128)), ("attb2", (128, 1)), ("attw3", (128, 1)),
                ("w1im", (128, 3, 16)), ("fusb1", (16, 1)),
                ("w2im", (128, 3, 8)), ("fusb2", (8, 1)),
                ("w3sel", (8, 8, 8)),
                ("onehot", (16, N)), ("id128", (128, 128)),
                ("gridT", (128, G)),
            ]:
                sb[name] = cpool.tile(list(shape), dr[name].dtype,
                                      name=f"sb_{name}")
                nc.sync.dma_start(out=sb[name][...], in_=dr[name][...])

            ones2 = cpool.tile([2, 1], F32R)
            nc.vector.memset(ones2[...].bitcast(F32), 1.0)
            ones16 = cpool.tile([1, 16], F32R)
            nc.vector.memset(ones16[...].bitcast(F32), 1.0)
            kerb3_t = cpool.tile([1, 1], F32)
            nc.vector.memset(kerb3_t[...], float(kerb3))
            nattb3_t = cpool.tile([1, 1], F32)
            nc.vector.memset(nattb3_t[...], float(-attb3))
            fusb3_t = cpool.tile([8, 1], F32)
            nc.vector.memset(fusb3_t[...], float(fusb3))

            # conv buffers allocated early; border memsets overlap early phases
            cvsb = ctx.enter_context(tc.tile_pool(name="conv_sbuf", bufs=1))
            # disjoint lifetimes share a slot: mpad dies once imY is built
            # (c2u reuses it)
            mpad = cvsb.tile([3, 66, 66], F32R, tag="cshare1")
            mTs = cvsb.tile([16, 2, 3, 128], F32R)
            c1p = cvsb.tile([16, 66, 66], F32R)
            imY = cvsb.tile([128, 64, 66], F32R)
            imY2 = cvsb.tile([48, 64, 66], F32R)
            c2u = cvsb.tile([8, 64, 64], F32R, tag="cshare1")
            sg = cvsb.tile([8, 512], F32)
            for t in (mpad, c1p):
                nc.vector.memset(t[:, 0:1, :].bitcast(F32), 0.0)     # top row
                nc.vector.memset(t[:, 65:66, :].bitcast(F32), 0.0)   # bottom row
                nc.vector.memset(t[:, 1:65, 0:1].bitcast(F32), 0.0)  # left col
                nc.vector.memset(t[:, 1:65, 65:66].bitcast(F32), 0.0)  # right col

            # ============ Phase 1: encoder + agg (feature-major) ============
            h1 = wpool.tile([64, 64], F32)
            h2 = wpool.tile([128, 64], F32)
            h3 = wpool.tile([128, 2, 64], F32)
            m = wpool.tile([128, 2, 16], F32)
            g1 = wpool.tile([128, 2, 16], F32)
            S = wpool.tile([128, 2, 16], F32R)

            with tc.tile_pool(name="pp1", bufs=4, space="PSUM") as pp1:
                ps = pp1.tile([64, 64], F32, tag="pp1t")
                nc.tensor.matmul(ps[...], sb["encw1"][...], sb["cpT"][...],
                                 start=True, stop=True)
                nc.scalar.activation(h1[...], ps[...], AF.Relu, bias=sb["encb1"][:, 0:1])

                ps2 = pp1.tile([128, 64], F32, tag="pp1t")
                nc.tensor.matmul(ps2[...], sb["encw2"][...], h1[...],
                                 start=True, stop=True)
                nc.scalar.activation(h2[...], ps2[...], AF.Relu, bias=sb["encb2"][:, 0:1])

                for fh in range(2):
                    ps3 = pp1.tile([128, 64], F32, tag="pp1t")
                    nc.tensor.matmul(ps3[...], sb["encw3"][:, 128 * fh : 128 * (fh + 1)],
                                     h2[...], start=True, stop=True)
                    nc.scalar.activation(h3[:, fh, :], ps3[...], AF.Relu,
                                         bias=sb["encb3"][:, fh : fh + 1])

                # mean over 4 control points (the 0.25 is folded into aggw1)
                h3r = h3[...].rearrange("p h (k c) -> p h k c", c=4)
                nc.vector.tensor_add(m[...], h3r[:, :, :, 0], h3r[:, :, :, 1])
                nc.vector.tensor_add(m[...], m[...], h3r[:, :, :, 2])
                nc.vector.tensor_add(m[...], m[...], h3r[:, :, :, 3])

                for dst, wname, bname, rhs in ((g1, "aggw1", "aggb1", m),
                                               (S, "aggw2", "aggb2", g1)):
                    for fh in range(2):
                        psg = pp1.tile([128, 16], F32, tag="pp1t")
                        for inh in range(2):
                            nc.tensor.matmul(psg[...], sb[wname][:, inh, fh, :],
                                             rhs[:, inh, :],
                                             start=(inh == 0), stop=(inh == 1))
                        nc.scalar.activation(dst[:, fh, :], psg[...], AF.Relu,
                                             bias=sb[bname][:, fh : fh + 1])

                # ============ Phase 2: Bezier points ============
                P = wpool.tile([2, N], F32)
                for k in range(K):
                    psb = pp1.tile([2, 100], F32, tag="pp1t")
                    nc.tensor.matmul(psb[...], sb["cpd"][:, 2 * k : 2 * k + 2],
                                     sb["basisT"][...], start=True, stop=True)
                    nc.vector.tensor_copy(P[:, R * k : R * (k + 1)], psb[...])

                pmin = wpool.tile([2, 1], F32)
                pmax = wpool.tile([2, 1], F32)
                rec = wpool.tile([2, 1], F32)
                nc.vector.tensor_reduce(pmin[...], P[...], axis=mybir.AxisListType.X,
                                        op=ALU.min)
                nc.vector.tensor_reduce(pmax[...], P[...], axis=mybir.AxisListType.X,
                                        op=ALU.max)
                nc.vector.tensor_tensor(rec[...], pmax[...], pmin[...], op=ALU.subtract)
                nc.vector.tensor_scalar_add(rec[...], rec[...], 1e-8)
                nc.vector.reciprocal(rec[...], rec[...])
                # pn = (P - pmin) * rec, in place
                nc.vector.tensor_scalar(P[...], P[...], pmin[...], rec[...],
                                        op0=ALU.subtract, op1=ALU.mult)

                P2 = wpool.tile([2, N], F32R)
                nc.vector.tensor_mul(P2[...], P[...], P[...])
                sqrow = wpool.tile([1, N], F32)
                for t0, w in NTILES_A + NTILES_B:
                    pss = pp1.tile([1, 512], F32, tag="pp1t")
                    nc.tensor.matmul(pss[:, :w], ones2[...], P2[:, t0 : t0 + w],
                                     start=True, stop=True)
                    nc.vector.tensor_copy(sqrow[:, t0 : t0 + w], pss[:, :w])

                # ---- ker MLP for all 3 scales -> cT [16, 3] ----
                e48 = wpool.tile([1, 48], F32)
                r48 = wpool.tile([1, 48], F32)
                cT = wpool.tile([16, 3], F32)
                k1 = wpool.tile([64, 16], F32)
                k2 = wpool.tile([32, 16], F32)
                for s in range(3):
                    psk1 = pp1.tile([64, 16], F32, tag="pp1t")
                    for inh in range(2):
                        nc.tensor.matmul(psk1[...], sb["kerw1"][:, inh, s, :],
                                         S[:, inh, :], start=(inh == 0), stop=(inh == 1))
                    nc.scalar.activation(k1[...], psk1[...], AF.Relu,
                                         bias=sb["kerb1"][:, s : s + 1])
                    psk2 = pp1.tile([32, 16], F32, tag="pp1t")
                    nc.tensor.matmul(psk2[...], sb["kerw2"][...], k1[...],
                                     start=True, stop=True)
                    nc.scalar.activation(k2[...], psk2[...], AF.Relu,
                                         bias=sb["kerb2"][:, 0:1])
                    psk3 = pp1.tile([1, 16], F32, tag="pp1t")
                    nc.tensor.matmul(psk3[...], sb["kerw3"][...], k2[...],
                                     start=True, stop=True)
                    # e48 k-major (col 3k+s) = exp(z + kerb3)
                    e48v = e48[...].rearrange("p (k s) -> p k s", s=3)
                    nc.scalar.activation(e48v[:, :, s], psk3[...],
                                         AF.Exp, bias=kerb3_t[...])

                # softplus, then c = 1/(2*sp^2) = (recip(sp)/sqrt(2))^2
                nc.vector.tensor_scalar_add(e48[...], e48[...], 1.0)
                nc.scalar.activation(e48[...], e48[...], AF.Ln)
                nc.vector.reciprocal(r48[...], e48[...])
                nc.scalar.activation(e48[...], r48[...], AF.Square,
                                     scale=0.7071067811865476)
                # transpose [1,48] -> [16k, 3s]
                nc.sync.dma_start(out=cT[...], in_=e48[...])

            # assembled point-side tensors
            xaug = wpool.tile([18, N], F32R)
            nc.sync.dma_start(out=xaug[0:16, :], in_=sb["onehot"][...])
            nc.sync.dma_start(out=xaug[16:18, :], in_=_r(P[...]))
            prep101 = wpool.tile([101, N], F32)
            nc.vector.memset(prep101[...], 1.0)
            for i in range(4):
                nc.sync.dma_start(out=prep101[32 * i : 32 * i + 2, :], in_=P[...])
                nc.sync.dma_start(out=prep101[32 * i + 3 : 32 * i + 4, :],
                                  in_=sqrow[...])

            # w1aug rows 16,17 = w1p for every scale (broadcast DMA)
            w1aug = wpool.tile([18, 3, 256], F32R)
            for s in range(3):
                nc.sync.dma_start(out=w1aug[16:18, s, :], in_=dr["w1p"][...])

            # ====== Phases 3+4 interleaved: per-scale att MLP + KDE ======
            rhs5 = [
                wpool.tile([101, N], BF16, name=f"rhs5_{s}", tag=f"rhs5_{s}")
                for s in range(3)
            ]
            m_all = wpool.tile([128, 3, 32], F32)

            with (
                tc.tile_pool(name="scale_work", bufs=1) as spool,
                tc.tile_pool(name="mlp_ps", bufs=4, space="PSUM") as pp3,
            ):
                # the 3 scales' chains are independent: emit stage-major so
                # the PE streams while DVE/ACT chase, no serial per-scale wall
                a1 = [spool.tile([128, 2, N], BF16, name=f"a1_{s}", tag=f"a1_{s}")
                      for s in range(3)]
                a2 = [spool.tile([128, N], F32R, name=f"a2_{s}", tag=f"a2_{s}")
                      for s in range(3)]
                esp = [spool.tile([1, N], BF16, name=f"esp_{s}", tag=f"esp_{s}")
                       for s in range(3)]
                cneg5 = [spool.tile([16, 101], F32R, name=f"cneg5_{s}",
                         tag=f"cneg5_{s}") for s in range(3)]
                for s in range(3):
                    pscf = pp3.tile([16, 256], F32, tag="mlp")
                    nc.tensor.matmul(pscf[...], S[:, 0, :], sb["aw1"][:, 0, s, :],
                                     start=True, stop=False)
                    nc.tensor.matmul(pscf[...], S[:, 1, :], sb["aw1"][:, 1, s, :],
                                     start=False, stop=False)
                    nc.tensor.matmul(pscf[...], ones16[...],
                                     sb["ab1row"][:, s, :], start=False, stop=True)
                    nc.vector.tensor_copy(_r(w1aug[0:16, s, :]), pscf[...])
                    nc.vector.tensor_scalar_mul(cneg5[s][...], _r(sb["mask101"][...]),
                                                cT[:, s : s + 1])
                for fh in range(2):
                    for t0, w in NTILES_A + NTILES_B:
                        for s in range(3):
                            psa = pp3.tile([128, 512], F32, tag="mlp")
                            nc.tensor.matmul(psa[:, :w],
                                             w1aug[:, s, 128 * fh : 128 * (fh + 1)],
                                             xaug[:, t0 : t0 + w],
                                             start=True, stop=True)
                            nc.vector.tensor_scalar_max(a1[s][:, fh, t0 : t0 + w],
                                                        psa[:, :w], 0.0)
                for t0, w in NTILES_A + NTILES_B:
                    for s in range(3):
                        psa2 = pp3.tile([128, 512], F32, tag="mlp")
                        for fh in range(2):
                            nc.tensor.matmul(psa2[:, :w], sb["attw2"][:, fh, :],
                                             a1[s][:, fh, t0 : t0 + w],
                                             start=(fh == 0), stop=(fh == 1))
                        nc.vector.tensor_scalar(a2[s][:, t0 : t0 + w], psa2[:, :w],
                                                sb["attb2"][:, 0:1], 0.0,
                                                op0=ALU.add, op1=ALU.max)
                for t0, w in NTILES_A + NTILES_B:
                    for s in range(3):
                        psz = pp3.tile([1, 512], F32, tag="mlp")
                        nc.tensor.matmul(psz[:, :w], sb["attw3"][...],
                                         a2[s][:, t0 : t0 + w], start=True, stop=True)
                        # exp(-(z + attb3))
                        nc.scalar.activation(esp[s][:, t0 : t0 + w], psz[:, :w],
                                             AF.Exp, bias=nattb3_t[...], scale=-1.0)
                for s in range(3):
                    nc.vector.tensor_scalar_add(esp[s][...], esp[s][...], 1.0)
                    nc.scalar.activation(esp[s][...], esp[s][...], AF.Ln)
                for t0, w in NTILES_A + NTILES_B:
                    for s in range(3):
                        psc = pp3.tile([101, 512], F32, tag="mlp")
                        nc.tensor.matmul(psc[:, :w], cneg5[s][...],
                                         sb["onehot"][:, t0 : t0 + w],
                                         start=True, stop=True)
                        nc.vector.tensor_mul(rhs5[s][:, t0 : t0 + w], psc[:, :w],
                                             prep101[:, t0 : t0 + w])
                # rows 32i+4 <- softplus(-z-b3)  (partition move via DMA)
                for s in range(3):
                    for i in range(4):
                        nc.sync.dma_start(out=rhs5[s][32 * i + 4 : 32 * i + 5, :],
                                          in_=esp[s][...])

            # ====== Phase 4: KDE (single exp per block, ACT accumulator) ====
            with (
                tc.tile_pool(name="kde_ps", bufs=2, space="PSUM") as kpp,
                tc.tile_pool(name="kde_scratch", bufs=2) as ksp,
            ):
                for s in range(3):
                    for gb in range(32):
                        kps = kpp.tile([128, 2048], F32, tag="kps")
                        for ti, (t0, w) in enumerate(NTILES_A + NTILES_B):
                            for j in range(4):
                                nc.tensor.matmul(
                                    kps[32 * j : 32 * (j + 1),
                                        512 * ti : 512 * ti + w],
                                    sb["gridT"][32 * ti : 32 * ti + 5,
                                               128 * gb + 32 * j : 128 * gb + 32 * (j + 1)],
                                    rhs5[s][32 * ti : 32 * ti + 5, t0 : t0 + w],
                                    start=True, stop=True,
                                    tile_position=(32 * ti, 32 * j),
                                )
                        scr = ksp.tile([128, N], BF16, tag="scr")
                        nc.scalar.activation(scr[...], kps[:, 0:N], AF.Exp,
                                             accum_out=m_all[:, s, gb : gb + 1])

                        # after each half of the grid, transpose the
                        # finished 16 gb-columns and scatter them so only the
                        # final half of scale 2 sits on the critical path.
                        if gb in (15, 31):
                            g0 = gb - 15
                            pst = kpp.tile([16, 128], F32, tag="kps", bufs=2)
                            nc.tensor.transpose(pst[...],
                                                m_all[:, s, g0 : g0 + 16],
                                                sb["id128"][...])
                            hb = g0 // 16
                            nc.vector.tensor_copy(mTs[:, hb, s, :], pst[...])
                            nc.sync.dma_start(
                                out=mpad[s : s + 1, 1 + 2 * g0 : 33 + 2 * g0,
                                         1:65].rearrange(
                                    "a (gb ph) x -> a gb ph x", ph=2),
                                in_=mTs[:, hb, s, :],
                            )
                            # im2col rows for this channel:
                            # imY[3dy+s, y, x] = mpad[s, y+dy, x]
                            y0, y1 = (0, 30) if g0 == 0 else (30, 64)
                            for dy in range(3):
                                nc.sync.dma_start(
                                    out=imY[3 * dy + s : 3 * dy + s + 1,
                                            y0:y1, :],
                                    in_=mpad[s : s + 1, y0 + dy : y1 + dy, :])

            # ============ Phase 5: conv head ============
            # replicate the 9 im2col rows into PE row groups 1..3 for 4-way
            # tile packing of conv1
            for q in range(1, 4):
                nc.scalar.dma_start(out=imY[32 * q : 32 * q + 9, :, :],
                                    in_=imY[0:9, :, :])
            with tc.tile_pool(name="conv_ps", bufs=4, space="PSUM") as cvp:
                for st in range(8):
                    q = st % 4
                    ps1 = cvp.tile([16, 512], F32, tag="cv1")
                    for dx in range(3):
                        nc.tensor.matmul(
                            ps1[...], sb["w1im"][32 * q : 32 * q + 9, dx, :],
                            imY[32 * q : 32 * q + 9,
                                st * 8 : st * 8 + 8, dx : dx + 64],
                            start=(dx == 0), stop=(dx == 2),
                            tile_position=(32 * q, 0),
                        )
                    nc.vector.tensor_scalar(c1p[:, 1 + st * 8 : 9 + st * 8, 1:65],
                                            ps1[...], sb["fusb1"][:, 0:1], 0.0,
                                            op0=ALU.add, op1=ALU.max)
                # imY2[16dy+c, y, x] = c1p[c, y+dy, x]; per-strip chunks
                # (issued on the idle ACT queue) so conv2 strips can start
                # while conv1 is still running
                for st in range(8):
                    for dy in range(3):
                        nc.scalar.dma_start(
                            out=imY2[16 * dy : 16 * dy + 16,
                                     st * 8 : st * 8 + 8, :],
                            in_=c1p[:, st * 8 + dy : st * 8 + 8 + dy, :])
                for st in range(8):
                    ps2c = cvp.tile([8, 512], F32, tag="cv2")
                    for dx in range(3):
                        nc.tensor.matmul(
                            ps2c[...], sb["w2im"][0:48, dx, :],
                            imY2[:, st * 8 : st * 8 + 8, dx : dx + 64],
                            start=(dx == 0), stop=(dx == 2),
                        )
                    nc.vector.tensor_scalar(c2u[:, st * 8 : 8 + st * 8, :],
                                            ps2c[...], sb["fusb2"][:, 0:1], 0.0,
                                            op0=ALU.add, op1=ALU.max)
            with tc.tile_pool(name="conv3_ps", bufs=1, space="PSUM") as cvp3:
                # 1x1 conv with strip-selector weights: psum row st = w3 . c2u strip st
                ps3c = cvp3.tile([8, 512], F32, tag="cv3")
                for st in range(8):
                    nc.tensor.matmul(ps3c[...],
                                     sb["w3sel"][:, st, :],
                                     c2u[:, st * 8 : st * 8 + 8, :],
                                     start=(st == 0), stop=(st == 7))
                nc.scalar.activation(sg[...], ps3c[...], AF.Sigmoid,
                                     bias=fusb3_t[...])

            # bn3 affine, then store
            nc.vector.tensor_scalar(sg[...], sg[...], bn3f, bn3b,
                                    op0=ALU.mult, op1=ALU.add)
            nc.sync.dma_start(
                out=out_dram[...].rearrange("a (r x) -> (a r) x", r=8),
                in_=sg[...])

        _split_multi_waits(nc)
        return nc

    return nc, finish


def _prepare_maps(inputs):
    """Host-side weight folding; returns per-core in_maps (list of dicts)."""
    f = {k: np.asarray(v, dtype=np.float32) for k, v in inputs.items()}
    basisT, onehot, gridT, mask101, id128 = _host_constants()

    bn1f = f["bn1_g"] / np.sqrt(np.float32(1.0 + BN_EPS))
    bn2f = f["bn2_g"] / np.sqrt(np.float32(1.0 + BN_EPS))
    A = (bn1f * bn2f).astype(np.float32)                     # [256]
    C = (f["bn1_b"] * bn2f + f["bn2_b"]).astype(np.float32)  # [256]

    scales = (0.5, 1.0, 2.0)
    kerw1 = np.stack(
        [(s * A)[:, None] * f["ker_w1"] for s in scales], 0
    )  # [3,256,64]
    kerb1 = np.stack(
        [s * (C @ f["ker_w1"]) + f["ker_b1"] for s in scales], 1
    )  # [64,3]
    aw1f = np.stack(
        [(s * A)[:, None] * f["att_w1"][:D] for s in scales], 0
    )  # [3,256,256]
    ab1row = np.stack(
        [s * (C @ f["att_w1"][:D]) + f["att_b1"] for s in scales], 0
    ).reshape(1, 3, 256)

    w1im9 = f["fus_w1"].transpose(2, 1, 3, 0).reshape(9, 3, 16)   # [3dy+c, dx, o]
    w1im = np.zeros((128, 3, 16), np.float32)
    for q in range(4):
        w1im[32 * q : 32 * q + 9] = w1im9                          # 4 row-group copies
    w2im48 = f["fus_w2"].transpose(2, 1, 3, 0).reshape(48, 3, 8)   # [16dy+c, dx, o]
    w2im = np.zeros((128, 3, 8), np.float32)
    for q in range(2):
        w2im[64 * q : 64 * q + 48] = w2im48                        # 2 row-group copies
    w3 = f["fus_w3"].reshape(8)
    w3sel = np.zeros((8, 8, 8), np.float32)                       # [c, st, r]
    for st in range(8):
        w3sel[:, st, st] = w3

    import ml_dtypes
    common = {
        "basisT": basisT,
        "onehot": onehot,
        "gridT": gridT.astype(ml_dtypes.bfloat16),
        "mask101": mask101,
        "id128": id128,
        "encw1": f["enc_w1"],
        "encb1": f["enc_b1"].reshape(64, 1),
        "encw2": f["enc_w2"],
        "encb2": f["enc_b2"].reshape(128, 1),
        "encw3": f["enc_w3"],
        "encb3": f["enc_b3"].reshape(2, 128).T.copy(),
        "aggw1": (0.25 * f["agg_w1"]).reshape(2, 128, 2, 128).transpose(1, 0, 2, 3).copy(),
        "aggb1": f["agg_b1"].reshape(2, 128).T.copy(),
        "aggw2": f["agg_w2"].reshape(2, 128, 2, 128).transpose(1, 0, 2, 3).copy(),
        "aggb2": f["agg_b2"].reshape(2, 128).T.copy(),
        "kerw1": kerw1.reshape(3, 2, 128, 64).transpose(2, 1, 0, 3).copy(),
        "kerb1": kerb1,
        "kerw2": f["ker_w2"],
        "kerb2": f["ker_b2"].reshape(32, 1),
        "kerw3": f["ker_w3"],
        "aw1": aw1f.reshape(3, 2, 128, 256).transpose(2, 1, 0, 3).copy(),
        "ab1row": ab1row,
        "w1p": f["att_w1"][D : D + 2].copy(),
        "attw2": f["att_w2"].reshape(2, 128, 128).transpose(1, 0, 2).astype(ml_dtypes.bfloat16),
        "attb2": f["att_b2"].reshape(128, 1),
        "attw3": f["att_w3"],
        "w1im": w1im,
        "fusb1": f["fus_b1"].reshape(16, 1),
        "w2im": w2im,
        "fusb2": f["fus_b2"].reshape(8, 1),
        "w3sel": w3sel,
    }
    common = {
        k: np.ascontiguousarray(v) if v.dtype == ml_dtypes.bfloat16
        else np.ascontiguousarray(v, dtype=np.float32)
        for k, v in common.items()
    }

    in_maps = []
    cp = f["control_points"]  # [8, 16, 4, 2]
    for c in range(NCORES):
        m = dict(common)
        m["cpT"] = np.ascontiguousarray(cp[c].reshape(64, 2).T)       # [2, 64]
        m["cpd"] = np.ascontiguousarray(cp[c].transpose(1, 0, 2).reshape(4, 32))
        in_maps.append(m)

    imm = dict(
        attb3=float(f["att_b3"][0]),
        kerb3=float(f["ker_b3"][0]),
        fusb3=float(f["fus_b3"][0]),
        bn3f=float(f["bn3_g"][0] / np.sqrt(1.0 + BN_EPS)),
        bn3b=float(f["bn3_b"][0]),
    )
    return in_maps, imm


def kernel(**inputs) -> np.ndarray:
    in_maps, imm = _prepare_maps(inputs)
    nc, finish = _build_program()
    nc = finish(**imm)
    res = run_bass_kernel_spmd(nc, in_maps, core_ids=list(range(NCORES)))
    kernel._last_results = res
    out = np.stack([r["out"].reshape(1, H, W) for r in res.results], axis=0)
    return out.astype(np.float32)


# revision 25
# speedup vs baseline: 1.0738x; 1.0279x over previous
"""Trainium2 Bass kernel for BezierParameterProcessor.

Data-parallel over the batch (character) axis: 1 character per NeuronCore, 8 cores.
All weights are host-prefolded (BN affines, per-scale multipliers, conv im2col
layout) and replicated to every core.

Device pipeline per character:
  1. encoder/agg MLPs (feature-major matmuls)        -> S [256f, 16k]
  2. Bezier points (16 small matmuls) + normalize    -> pn [2, 1600]
  3. ker MLP for all scales -> c_k = 1/(2*softplus^2)
  4. per scale s: att MLP -> softplus(-z); assemble
     rhs5_s = [c*x; c*y; -c; -c*|p|^2; softplus(-z)] (bf16), then the KDE
     grid loop for s: 16-way PE-tiled bf16 matmuls -> (-c*d2 + ln attn) in
     PSUM (split across two chunk tiles A/B), Exp on ACT, row-sums split
     between GPSIMD (halving add) and DVE (reduces).  The att MLP of scale
     s+1 executes in the PE/DVE shadow of scale s's ACT-bound KDE loop.
     After each scale: PE-transpose of the map, scatter into the padded
     channel-major conv input.
  5. conv head: im2col row shifts via contiguous SBUF DMAs, f32r tap
     matmuls, sigmoid+bn3 on a [8,512] layout.
"""

import sys

sys.path.insert(0, "/opt/trn_rl_repo")

import numpy as np
from math import comb
from contextlib import ExitStack

import concourse.bass as bass
import concourse.tile as tile
from concourse import mybir
from concourse.bass_utils import run_bass_kernel_spmd

F32 = mybir.dt.float32
F32R = mybir.dt.float32r
BF16 = mybir.dt.bfloat16
AF = mybir.ActivationFunctionType
ALU = mybir.AluOpType

B, K, R, D = 8, 16, 100, 256
N = K * R            # 1600
H = W = 64
G = H * W            # 4096
NCORES = 8
BN_EPS = 1e-5
NTILES_A = [(0, 512), (512, 512)]          # psum chunk A: n-cols 0..1023
NTILES_B = [(1024, 512), (1536, 64)]       # psum chunk B: n-cols 1024..1599
NB = 576                                   # chunk B width


def _r(ap):
    return ap.bitcast(F32R)


def _host_constants():
    t = np.linspace(0.0, 1.0, R).astype(np.float64)
    basisT = np.stack(
        [comb(3, c) * t**c * (1.0 - t) ** (3 - c) for c in range(4)], axis=0
    ).astype(np.float32)                               # [4, 100]

    onehot = np.zeros((K, N), np.float32)
    for k in range(K):
        onehot[k, k * R : (k + 1) * R] = 1.0           # [16, 1600]

    xs = np.linspace(0.0, 1.0, W).astype(np.float64)
    gx = np.tile(xs, H)                                 # g % 64
    gy = np.repeat(xs, W)                               # g // 64
    grid5 = np.stack(
        [2.0 * gx, 2.0 * gy, gx**2 + gy**2, np.ones(G), -np.ones(G)], axis=0
    ).astype(np.float32)                                # [5, 4096]
    # replicated at partition offsets 0/32/64/96 for 16-way PE tile packing
    gridT = np.zeros((128, G), np.float32)
    for i in range(4):
        gridT[32 * i : 32 * i + 5, :] = grid5
    # sign mask for building rhs rows, replicated at the same offsets
    mask101 = np.zeros((16, 101), np.float32)
    pat = np.array([1.0, 1.0, -1.0, -1.0, 0.0], np.float32)
    for i in range(4):
        mask101[:, 32 * i : 32 * i + 5] = pat[None, :]
    id128 = np.eye(128, dtype=np.float32)
    return basisT, onehot, gridT, mask101, id128


def _split_multi_waits(nc):
    """Walrus codegen in this toolchain accepts one sync-wait per instruction;
    carry extra waits on same-engine NoOps inserted just before."""
    for f in nc.m.functions:
        for blk in f.blocks:
            idx = 0
            while idx < len(blk.instructions):
                inst = blk.instructions[idx]
                si = inst.sync_info
                if si is not None and len(si.on_wait) > 1:
                    waits = list(si.on_wait)
                    for j, w in enumerate(waits[:-1]):
                        nop = mybir.InstNoOp(name=f"WSPLIT-{nc.next_id()}",
                                             ins=[], outs=[])
                        nop.engine = inst.engine
                        nop.sync_info = mybir.SyncInfo(on_wait=[w], on_update=[])
                        blk.instructions.insert(idx + j, nop)
                    idx += len(waits) - 1
                    inst.sync_info = mybir.SyncInfo(on_wait=[waits[-1]],
                                                    on_update=list(si.on_update))
                idx += 1


def _build_program():
    nc = bass.Bass()

    # ---- DRAM I/O declarations (shapes only; data supplied per core) ----
    dr = {}

    def din(name, shape, dt=F32):
        dr[name] = nc.dram_tensor(name, list(shape), dt, kind="ExternalInput")
        return dr[name]

    din("cpT", (2, 64))            # encoder input, cols n=(k,cp)
    din("cpd", (4, 32))            # bezier lhsT, cols 2k+d
    din("basisT", (4, 100))
    din("onehot", (16, N), F32R)
    din("gridT", (128, G), BF16)
    din("mask101", (16, 101))
    din("id128", (128, 128))
    din("encw1", (2, 64)), din("encb1", (64, 1))
    din("encw2", (64, 128)), din("encb2", (128, 1))
    din("encw3", (128, 256)), din("encb3", (128, 2))
    din("aggw1", (128, 2, 2, 128)), din("aggb1", (128, 2))
    din("aggw2", (128, 2, 2, 128)), din("aggb2", (128, 2))
    din("kerw1", (128, 2, 3, 64), F32R), din("kerb1", (64, 3))
    din("kerw2", (64, 32)), din("kerb2", (32, 1))
    din("kerw3", (32, 1))
    din("aw1", (128, 2, 3, 256), F32R), din("ab1row", (1, 3, 256), F32R)
    din("w1p", (2, 256), F32R)
    din("attw2", (128, 2, 128), BF16), din("attb2", (128, 1))
    din("attw3", (128, 1), F32R)
    din("w1im", (128, 3, 16), F32R), din("fusb1", (16, 1))
    din("w2im", (128, 3, 8), F32R), din("fusb2", (8, 1))
    din("w3sel", (8, 8, 8), F32R)
    out_dram = nc.dram_tensor("out", [1, G], F32, kind="ExternalOutput")

    # imm scalars get baked at build time from the actual inputs:
    # we return a closure that finishes the build given those values.
    def finish(attb3, kerb3, fusb3, bn3f, bn3b):
        with ExitStack() as ctx:
            tc = ctx.enter_context(tile.TileContext(nc))
            cpool = ctx.enter_context(tc.tile_pool(name="consts", bufs=1))
            wpool = ctx.enter_context(tc.tile_pool(name="work", bufs=1))

            # ---- load constants/weights to SBUF (big late-use ones last) ----
            sb = {}
            for name, shape in [
                ("cpT", (2, 64)), ("cpd", (4, 32)), ("basisT", (4, 100)),
                ("mask101", (16, 101)),
                ("encw1", (2, 64)), ("encb1", (64, 1)),
                ("encw2", (64, 128)), ("encb2", (128, 1)),
                ("encw3", (128, 256)), ("encb3", (128, 2)),
                ("aggw1", (128, 2, 2, 128)), ("aggb1", (128, 2)),
                ("aggw2", (128, 2, 2, 128)), ("aggb2", (128, 2)),
                ("kerw1", (128, 2, 3, 64)), ("kerb1", (64, 3)),
                ("kerw2", (64, 32)), ("kerb2", (32, 1)),
                ("kerw3", (32, 1)),
                ("aw1", (128, 2, 3, 256)), ("ab1row", (1, 3, 256)),
                ("attw2", (128, 2, 128)), ("attb2", (128, 1)), ("attw3", (128, 1)),
                ("w1im", (128, 3, 16)), ("fusb1", (16, 1)),
                ("w2im", (128, 3, 8)), ("fusb2", (8, 1)),
                ("w3sel", (8, 8, 8)),
                ("onehot", (16, N)), ("id128", (128, 128)),
                ("gridT", (128, G)),
            ]:
                sb[name] = cpool.tile(list(shape), dr[name].dtype,
                                      name=f"sb_{name}")
                nc.sync.dma_start(out=sb[name][...], in_=dr[name][...])

            ones2 = cpool.tile([2, 1], F32R)
            nc.vector.memset(ones2[...].bitcast(F32), 1.0)
            ones16 = cpool.tile([1, 16], F32R)
            nc.vector.memset(ones16[...].bitcast(F32), 1.0)
            kerb3_t = cpool.tile([1, 1], F32)
            nc.vector.memset(kerb3_t[...], float(kerb3))
            nattb3_t = cpool.tile([1, 1], F32)
            nc.vector.memset(nattb3_t[...], float(-attb3))
            fusb3_t = cpool.tile([8, 1], F32)
            nc.vector.memset(fusb3_t[...], float(fusb3))

            # conv buffers allocated early; border memsets overlap early phases
            cvsb = ctx.enter_context(tc.tile_pool(name="conv_sbuf", bufs=1))
            # disjoint lifetimes share a slot: mpad dies once imY is built
            # (c2u reuses it)
            mpad = cvsb.tile([3, 66, 66], F32R, tag="cshare1")
            mTs = cvsb.tile([16, 2, 3, 128], F32R)
            c1p = cvsb.tile([16, 66, 66], F32R)
            imY = cvsb.tile([128, 64, 66], F32R)
            imY2 = cvsb.tile([48, 64, 66], F32R)
            c2u = cvsb.tile([8, 64, 64], F32R, tag="cshare1")
            sg = cvsb.tile([8, 512], F32)
            for t in (mpad, c1p):
                nc.vector.memset(t[:, 0:1, :].bitcast(F32), 0.0)     # top row
                nc.vector.memset(t[:, 65:66, :].bitcast(F32), 0.0)   # bottom row
                nc.vector.memset(t[:, 1:65, 0:1].bitcast(F32), 0.0)  # left col
                nc.vector.memset(t[:, 1:65, 65:66].bitcast(F32), 0.0)  # right col

            # ============ Phase 1: encoder + agg (feature-major) ============
            h1 = wpool.tile([64, 64], F32)
            h2 = wpool.tile([128, 64], F32)
            h3 = wpool.tile([128, 2, 64], F32)
            m = wpool.tile([128, 2, 16], F32)
            g1 = wpool.tile([128, 2, 16], F32)
            S = wpool.tile([128, 2, 16], F32R)

            with tc.tile_pool(name="pp1", bufs=4, space="PSUM") as pp1:
                ps = pp1.tile([64, 64], F32, tag="pp1t")
                nc.tensor.matmul(ps[...], sb["encw1"][...], sb["cpT"][...],
                                 start=True, stop=True)
                nc.scalar.activation(h1[...], ps[...], AF.Relu, bias=sb["encb1"][:, 0:1])

                ps2 = pp1.tile([128, 64], F32, tag="pp1t")
                nc.tensor.matmul(ps2[...], sb["encw2"][...], h1[...],
                                 start=True, stop=True)
                nc.scalar.activation(h2[...], ps2[...], AF.Relu, bias=sb["encb2"][:, 0:1])

                for fh in range(2):
                    ps3 = pp1.tile([128, 64], F32, tag="pp1t")
                    nc.tensor.matmul(ps3[...], sb["encw3"][:, 128 * fh : 128 * (fh + 1)],
                                     h2[...], start=True, stop=True)
                    nc.scalar.activation(h3[:, fh, :], ps3[...], AF.Relu,
                                         bias=sb["encb3"][:, fh : fh + 1])

                # mean over 4 control points (the 0.25 is folded into aggw1)
                h3r = h3[...].rearrange("p h (k c) -> p h k c", c=4)
                nc.vector.tensor_add(m[...], h3r[:, :, :, 0], h3r[:, :, :, 1])
                nc.vector.tensor_add(m[...], m[...], h3r[:, :, :, 2])
                nc.vector.tensor_add(m[...], m[...], h3r[:, :, :, 3])

                for dst, wname, bname, rhs in ((g1, "aggw1", "aggb1", m),
                                               (S, "aggw2", "aggb2", g1)):
                    for fh in range(2):
                        psg = pp1.tile([128, 16], F32, tag="pp1t")
                        for inh in range(2):
                            nc.tensor.matmul(psg[...], sb[wname][:, inh, fh, :],
                                             rhs[:, inh, :],
                                             start=(inh == 0), stop=(inh == 1))
                        nc.scalar.activation(dst[:, fh, :], psg[...], AF.Relu,
                                             bias=sb[bname][:, fh : fh + 1])

                # ============ Phase 2: Bezier points ============
                P = wpool.tile([2, N], F32)
                for k in range(K):
                    psb = pp1.tile([2, 100], F32, tag="pp1t")
                    nc.tensor.matmul(psb[...], sb["cpd"][:, 2 * k : 2 * k + 2],
                                     sb["basisT"][...], start=True, stop=True)
                    nc.vector.tensor_copy(P[:, R * k : R * (k + 1)], psb[...])

                pmin = wpool.tile([2, 1], F32)
                pmax = wpool.tile([2, 1], F32)
                rec = wpool.tile([2, 1], F32)
                nc.vector.tensor_reduce(pmin[...], P[...], axis=mybir.AxisListType.X,
                                        op=ALU.min)
                nc.vector.tensor_reduce(pmax[...], P[...], axis=mybir.AxisListType.X,
                                        op=ALU.max)
                nc.vector.tensor_tensor(rec[...], pmax[...], pmin[...], op=ALU.subtract)
                nc.vector.tensor_scalar_add(rec[...], rec[...], 1e-8)
                nc.vector.reciprocal(rec[...], rec[...])
                # pn = (P - pmin) * rec, in place
                nc.vector.tensor_scalar(P[...], P[...], pmin[...], rec[...],
                                        op0=ALU.subtract, op1=ALU.mult)

                P2 = wpool.tile([2, N], F32R)
                nc.vector.tensor_mul(P2[...], P[...], P[...])
                sqrow = wpool.tile([1, N], F32)
                for t0, w in NTILES_A + NTILES_B:
                    pss = pp1.tile([1, 512], F32, tag="pp1t")
                    nc.tensor.matmul(pss[:, :w], ones2[...], P2[:, t0 : t0 + w],
                                     start=True, stop=True)
                    nc.vector.tensor_copy(sqrow[:, t0 : t0 + w], pss[:, :w])

                # ---- ker MLP for all 3 scales -> cT [16, 3] ----
                e48 = wpool.tile([1, 48], F32)
                r48 = wpool.tile([1, 48], F32)
                cT = wpool.tile([16, 3], F32)
                k1 = wpool.tile([64, 16], F32)
                k2 = wpool.tile([32, 16], F32)
                for s in range(3):
                    psk1 = pp1.tile([64, 16], F32, tag="pp1t")
                    for inh in range(2):
                        nc.tensor.matmul(psk1[...], sb["kerw1"][:, inh, s, :],
                                         S[:, inh, :], start=(inh == 0), stop=(inh == 1))
                    nc.scalar.activation(k1[...], psk1[...], AF.Relu,
                                         bias=sb["kerb1"][:, s : s + 1])
                    psk2 = pp1.tile([32, 16], F32, tag="pp1t")
                    nc.tensor.matmul(psk2[...], sb["kerw2"][...], k1[...],
                                     start=True, stop=True)
                    nc.scalar.activation(k2[...], psk2[...], AF.Relu,
                                         bias=sb["kerb2"][:, 0:1])
                    psk3 = pp1.tile([1, 16], F32, tag="pp1t")
                    nc.tensor.matmul(psk3[...], sb["kerw3"][...], k2[...],
                                     start=True, stop=True)
                    # e48 k-major (col 3k+s) = exp(z + kerb3)
                    e48v = e48[...].rearrange("p (k s) -> p k s", s=3)
                    nc.scalar.activation(e48v[:, :, s], psk3[...],
                                         AF.Exp, bias=kerb3_t[...])

                # softplus, then c = 1/(2*sp^2) = (recip(sp)/sqrt(2))^2
                nc.vector.tensor_scalar_add(e48[...], e48[...], 1.0)
                nc.scalar.activation(e48[...], e48[...], AF.Ln)
                nc.vector.reciprocal(r48[...], e48[...])
                nc.scalar.activation(e48[...], r48[...], AF.Square,
                                     scale=0.7071067811865476)
                # transpose [1,48] -> [16k, 3s]
                nc.sync.dma_start(out=cT[...], in_=e48[...])

            # assembled point-side tensors
            xaug = wpool.tile([18, N], F32R)
            nc.sync.dma_start(out=xaug[0:16, :], in_=sb["onehot"][...])
            nc.sync.dma_start(out=xaug[16:18, :], in_=_r(P[...]))
            prep101 = wpool.tile([101, N], F32)
            nc.vector.memset(prep101[...], 1.0)
            for i in range(4):
                nc.sync.dma_start(out=prep101[32 * i : 32 * i + 2, :], in_=P[...])
                nc.sync.dma_start(out=prep101[32 * i + 3 : 32 * i + 4, :],
                                  in_=sqrow[...])

            # w1aug rows 16,17 = w1p for every scale (broadcast DMA)
            w1aug = wpool.tile([18, 3, 256], F32R)
            for s in range(3):
                nc.sync.dma_start(out=w1aug[16:18, s, :], in_=dr["w1p"][...])

            # ====== Phases 3+4 interleaved: per-scale att MLP + KDE ======
            rhs5 = [
                wpool.tile([101, N], BF16, name=f"rhs5_{s}", tag=f"rhs5_{s}")
                for s in range(3)
            ]
            m_all = wpool.tile([128, 3, 32], F32)

            with (
                tc.tile_pool(name="scale_work", bufs=1) as spool,
                tc.tile_pool(name="mlp_ps", bufs=4, space="PSUM") as pp3,
            ):
                # the 3 scales' chains are independent: emit stage-major so
                # the PE streams while DVE/ACT chase, no serial per-scale wall
                a1 = [spool.tile([128, 2, N], BF16, name=f"a1_{s}", tag=f"a1_{s}")
                      for s in range(3)]
                a2 = [spool.tile([128, N], F32R, name=f"a2_{s}", tag=f"a2_{s}")
                      for s in range(3)]
                esp = [spool.tile([1, N], BF16, name=f"esp_{s}", tag=f"esp_{s}")
                       for s in range(3)]
                cneg5 = [spool.tile([16, 101], F32R, name=f"cneg5_{s}",
                         tag=f"cneg5_{s}") for s in range(3)]
                for s in range(3):
                    pscf = pp3.tile([16, 256], F32, tag="mlp")
                    nc.tensor.matmul(pscf[...], S[:, 0, :], sb["aw1"][:, 0, s, :],
                                     start=True, stop=False)
                    nc.tensor.matmul(pscf[...], S[:, 1, :], sb["aw1"][:, 1, s, :],
                                     start=False, stop=False)
                    nc.tensor.matmul(pscf[...], ones16[...],
                                     sb["ab1row"][:, s, :], start=False, stop=True)
                    nc.vector.tensor_copy(_r(w1aug[0:16, s, :]), pscf[...])
                    nc.vector.tensor_scalar_mul(cneg5[s][...], _r(sb["mask101"][...]),
                                                cT[:, s : s + 1])
                for fh in range(2):
                    for t0, w in NTILES_A + NTILES_B:
                        for s in range(3):
                            psa = pp3.tile([128, 512], F32, tag="mlp")
                            nc.tensor.matmul(psa[:, :w],
                                             w1aug[:, s, 128 * fh : 128 * (fh + 1)],
                                             xaug[:, t0 : t0 + w],
                                             start=True, stop=True)
                            nc.vector.tensor_scalar_max(a1[s][:, fh, t0 : t0 + w],
                                                        psa[:, :w], 0.0)
                for t0, w in NTILES_A + NTILES_B:
                    for s in range(3):
                        psa2 = pp3.tile([128, 512], F32, tag="mlp")
                        for fh in range(2):
                            nc.tensor.matmul(psa2[:, :w], sb["attw2"][:, fh, :],
                                             a1[s][:, fh, t0 : t0 + w],
                                             start=(fh == 0), stop=(fh == 1))
                        nc.vector.tensor_scalar(a2[s][:, t0 : t0 + w], psa2[:, :w],
                                                sb["attb2"][:, 0:1], 0.0,
                                                op0=ALU.add, op1=ALU.max)
                for t0, w in NTILES_A + NTILES_B:
                    for s in range(3):
                        psz = pp3.tile([1, 512], F32, tag="mlp")
                        nc.tensor.matmul(psz[:, :w], sb["attw3"][...],
                                         a2[s][:, t0 : t0 + w], start=True, stop=True)
                        # exp(-(z + attb3))
                        nc.scalar.activation(esp[s][:, t0 : t0 + w], psz[:, :w],
                                             AF.Exp, bias=nattb3_t[...], scale=-1.0)
                for s in range(3):
                    nc.vector.tensor_scalar_add(esp[s][...], esp[s][...], 1.0)
                    nc.scalar.activation(esp[s][...], esp[s][...], AF.Ln)
                for t0, w in NTILES_A + NTILES_B:
                    for s in range(3):
                        psc = pp3.tile([101, 512], F32, tag="mlp")
                        nc.tensor.matmul(psc[:, :w], cneg5[s][...],
                                         sb["onehot"][:, t0 : t0 + w],
                                         start=True, stop=True)
                        nc.vector.tensor_mul(rhs5[s][:, t0 : t0 + w], psc[:, :w],
                                             prep101[:, t0 : t0 + w])
                # rows 32i+4 <- softplus(-z-b3)  (partition move via DMA)
                for s in range(3):
                    for i in range(4):
                        nc.sync.dma_start(out=rhs5[s][32 * i + 4 : 32 * i + 5, :],
                                          in_=esp[s][...])

            # ====== Phase 4: KDE (single exp per block, ACT accumulator) ====
            with (
                tc.tile_pool(name="kde_ps", bufs=2, space="PSUM") as kpp,
                tc.tile_pool(name="kde_scratch", bufs=2) as ksp,
            ):
                for s in range(3):
                    for gb in range(32):
                        kps = kpp.tile([128, 2048], F32, tag="kps")
                        for ti, (t0, w) in enumerate(NTILES_A + NTILES_B):
                            for j in range(4):
                                nc.tensor.matmul(
                                    kps[32 * j : 32 * (j + 1),
                                        512 * ti : 512 * ti + w],
                                    sb["gridT"][32 * ti : 32 * ti + 5,
                                               128 * gb + 32 * j : 128 * gb + 32 * (j + 1)],
                                    rhs5[s][32 * ti : 32 * ti + 5, t0 : t0 + w],
                                    start=True, stop=True,
                                    tile_position=(32 * ti, 32 * j),
                                )
                        scr = ksp.tile([128, N], BF16, tag="scr")
                        nc.scalar.activation(scr[...], kps[:, 0:N], AF.Exp,
                                             accum_out=m_all[:, s, gb : gb + 1])

                        # after each half of the grid, transpose the
                        # finished 16 gb-columns and scatter them so only the
                        # final half of scale 2 sits on the critical path.
                        if gb in (15, 31):
                            g0 = gb - 15
                            pst = kpp.tile([16, 128], F32, tag="kps", bufs=2)
                            nc.tensor.transpose(pst[...],
                                                m_all[:, s, g0 : g0 + 16],
                                                sb["id128"][...])
                            hb = g0 // 16
                            nc.vector.tensor_copy(mTs[:, hb, s, :], pst[...])
                            nc.sync.dma_start(
                                out=mpad[s : s + 1, 1 + 2 * g0 : 33 + 2 * g0,
                                         1:65].rearrange(
                                    "a (gb ph) x -> a gb ph x", ph=2),
                                in_=mTs[:, hb, s, :],
                            )
                            # im2col rows for this channel:
                            # imY[3dy+s, y, x] = mpad[s, y+dy, x]
                            y0, y1 = (0, 30) if g0 == 0 else (30, 64)
                            for dy in range(3):
                                nc.sync.dma_start(
                                    out=imY[3 * dy + s : 3 * dy + s + 1,
                                            y0:y1, :],
                                    in_=mpad[s : s + 1, y0 + dy : y1 + dy, :])

            # ============ Phase 5: conv head ============
            with tc.tile_pool(name="conv_ps", bufs=4, space="PSUM") as cvp:
                for st in range(8):
                    ps1 = cvp.tile([16, 512], F32, tag="cv1")
                    for dx in range(3):
                        nc.tensor.matmul(
                            ps1[...], sb["w1im"][0:9, dx, :],
                            imY[0:9, st * 8 : st * 8 + 8, dx : dx + 64],
                            start=(dx == 0), stop=(dx == 2),
                        )
                    nc.vector.tensor_scalar(c1p[:, 1 + st * 8 : 9 + st * 8, 1:65],
                                            ps1[...], sb["fusb1"][:, 0:1], 0.0,
                                            op0=ALU.add, op1=ALU.max)
                # imY2[16dy+c, y, x] = c1p[c, y+dy, x]; per-strip chunks
                # (issued on the idle ACT queue) so conv2 strips can start
                # while conv1 is still running
                for st in range(8):
                    for dy in range(3):
                        nc.scalar.dma_start(
                            out=imY2[16 * dy : 16 * dy + 16,
                                     st * 8 : st * 8 + 8, :],
                            in_=c1p[:, st * 8 + dy : st * 8 + 8 + dy, :])
                for st in range(8):
                    ps2c = cvp.tile([8, 512], F32, tag="cv2")
                    for dx in range(3):
                        nc.tensor.matmul(
                            ps2c[...], sb["w2im"][0:48, dx, :],
                            imY2[:, st * 8 : st * 8 + 8, dx : dx + 64],
                            start=(dx == 0), stop=(dx == 2),
                        )
                    nc.vector.tensor_scalar(c2u[:, st * 8 : 8 + st * 8, :],
                                            ps2c[...], sb["fusb2"][:, 0:1], 0.0,
                                            op0=ALU.add, op1=ALU.max)
            with tc.tile_pool(name="conv3_ps", bufs=1, space="PSUM") as cvp3:
                # 1x1 conv with strip-selector weights: psum row st = w3 . c2u strip st
                ps3c = cvp3.tile([8, 512], F32, tag="cv3")
                for st in range(8):
                    nc.tensor.matmul(ps3c[...],
                                     sb["w3sel"][:, st, :],
                                     c2u[:, st * 8 : st * 8 + 8, :],
                                     start=(st == 0), stop=(st == 7))
                nc.scalar.activation(sg[...], ps3c[...], AF.Sigmoid,
                                     bias=fusb3_t[...])

            # bn3 affine, then store
            nc.vector.tensor_scalar(sg[...], sg[...], bn3f, bn3b,
                                    op0=ALU.mult, op1=ALU.add)
            nc.sync.dma_start(
                out=out_dram[...].rearrange("a (r x) -> (a r) x", r=8),
                in_=sg[...])

        _split_multi_waits(nc)
        return nc

    return nc, finish


def _prepare_maps(inputs):
    """Host-side weight folding; returns per-core in_maps (list of dicts)."""
    f = {k: np.asarray(v, dtype=np.float32) for k, v in inputs.items()}
    basisT, onehot, gridT, mask101, id128 = _host_constants()

    bn1f = f["bn1_g"] / np.sqrt(np.float32(1.0 + BN_EPS))
    bn2f = f["bn2_g"] / np.sqrt(np.float32(1.0 + BN_EPS))
    A = (bn1f * bn2f).astype(np.float32)                     # [256]
    C = (f["bn1_b"] * bn2f + f["bn2_b"]).astype(np.float32)  # [256]

    scales = (0.5, 1.0, 2.0)
    kerw1 = np.stack(
        [(s * A)[:, None] * f["ker_w1"] for s in scales], 0
    )  # [3,256,64]
    kerb1 = np.stack(
        [s * (C @ f["ker_w1"]) + f["ker_b1"] for s in scales], 1
    )  # [64,3]
    aw1f = np.stack(
        [(s * A)[:, None] * f["att_w1"][:D] for s in scales], 0
    )  # [3,256,256]
    ab1row = np.stack(
        [s * (C @ f["att_w1"][:D]) + f["att_b1"] for s in scales], 0
    ).reshape(1, 3, 256)

    w1im9 = f["fus_w1"].transpose(2, 1, 3, 0).reshape(9, 3, 16)   # [3dy+c, dx, o]
    w1im = np.zeros((128, 3, 16), np.float32)
    for q in range(4):
        w1im[32 * q : 32 * q + 9] = w1im9                          # 4 row-group copies
    w2im48 = f["fus_w2"].transpose(2, 1, 3, 0).reshape(48, 3, 8)   # [16dy+c, dx, o]
    w2im = np.zeros((128, 3, 8), np.float32)
    for q in range(2):
        w2im[64 * q : 64 * q + 48] = w2im48                        # 2 row-group copies
    w3 = f["fus_w3"].reshape(8)
    w3sel = np.zeros((8, 8, 8), np.float32)                       # [c, st, r]
    for st in range(8):
        w3sel[:, st, st] = w3

    import ml_dtypes
    common = {
        "basisT": basisT,
        "onehot": onehot,
        "gridT": gridT.astype(ml_dtypes.bfloat16),
        "mask101": mask101,
        "id128": id128,
        "encw1": f["enc_w1"],
        "encb1": f["enc_b1"].reshape(64, 1),
        "encw2": f["enc_w2"],
        "encb2": f["enc_b2"].reshape(128, 1),
        "encw3": f["enc_w3"],
        "encb3": f["enc_b3"].reshape(2, 128).T.copy(),
        "aggw1": (0.25 * f["agg_w1"]).reshape(2, 128, 2, 128).transpose(1, 0, 2, 3).copy(),
        "aggb1": f["agg_b1"].reshape(2, 128).T.copy(),
        "aggw2": f["agg_w2"].reshape(2, 128, 2, 128).transpose(1, 0, 2, 3).copy(),
        "aggb2": f["agg_b2"].reshape(2, 128).T.copy(),
        "kerw1": kerw1.reshape(3, 2, 128, 64).transpose(2, 1, 0, 3).copy(),
        "kerb1": kerb1,
        "kerw2": f["ker_w2"],
        "kerb2": f["ker_b2"].reshape(32, 1),
        "kerw3": f["ker_w3"],
        "aw1": aw1f.reshape(3, 2, 128, 256).transpose(2, 1, 0, 3).copy(),
        "ab1row": ab1row,
        "w1p": f["att_w1"][D : D + 2].copy(),
        "attw2": f["att_w2"].reshape(2, 128, 128).transpose(1, 0, 2).astype(ml_dtypes.bfloat16),
        "attb2": f["att_b2"].reshape(128, 1),
        "attw3": f["att_w3"],
        "w1im": w1im,
        "fusb1": f["fus_b1"].reshape(16, 1),
        "w2im": w2im,
        "fusb2": f["fus_b2"].reshape(8, 1),
        "w3sel": w3sel,
    }
    common = {
        k: np.ascontiguousarray(v) if v.dtype == ml_dtypes.bfloat16
        else np.ascontiguousarray(v, dtype=np.float32)
        for k, v in common.items()
    }

    in_maps = []
    cp = f["control_points"]  # [8, 16, 4, 2]
    for c in range(NCORES):
        m = dict(common)
        m["cpT"] = np.ascontiguousarray(cp[c].reshape(64, 2).T)       # [2, 64]
        m["cpd"] = np.ascontiguousarray(cp[c].transpose(1, 0, 2).reshape(4, 32))
        in_maps.append(m)

    imm = dict(
        attb3=float(f["att_b3"][0]),
        kerb3=float(f["ker_b3"][0]),
        fusb3=float(f["fus_b3"][0]),
        bn3f=float(f["bn3_g"][0] / np.sqrt(1.0 + BN_EPS)),
        bn3b=float(f["bn3_b"][0]),
    )
    return in_maps, imm


def kernel(**inputs) -> np.ndarray:
    in_maps, imm = _prepare_maps(inputs)
    nc, finish = _build_program()
    nc = finish(**imm)
    res = run_bass_kernel_spmd(nc, in_maps, core_ids=list(range(NCORES)))
    kernel._last_results = res
    out = np.stack([r["out"].reshape(1, H, W) for r in res.results], axis=0)
    return out.astype(np.float32)


# revision 27
# speedup vs baseline: 1.1010x; 1.0253x over previous
"""Trainium2 Bass kernel for BezierParameterProcessor.

Data-parallel over the batch (character) axis: 1 character per NeuronCore, 8 cores.
All weights are host-prefolded (BN affines, per-scale multipliers, conv im2col
layout) and replicated to every core.

Device pipeline per character:
  1. encoder/agg MLPs (feature-major matmuls)        -> S [256f, 16k]
  2. Bezier points (16 small matmuls) + normalize    -> pn [2, 1600]
  3. ker MLP for all scales -> c_k = 1/(2*softplus^2)
  4. per scale s: att MLP -> softplus(-z); assemble
     rhs5_s = [c*x; c*y; -c; -c*|p|^2; softplus(-z)] (bf16), then the KDE
     grid loop for s: 16-way PE-tiled bf16 matmuls -> (-c*d2 + ln attn) in
     PSUM (split across two chunk tiles A/B), Exp on ACT, row-sums split
     between GPSIMD (halving add) and DVE (reduces).  The att MLP of scale
     s+1 executes in the PE/DVE shadow of scale s's ACT-bound KDE loop.
     After each scale: PE-transpose of the map, scatter into the padded
     channel-major conv input.
  5. conv head: im2col row shifts via contiguous SBUF DMAs, f32r tap
     matmuls, sigmoid+bn3 on a [8,512] layout.
"""

import sys

sys.path.insert(0, "/opt/trn_rl_repo")

import numpy as np
from math import comb
from contextlib import ExitStack

import concourse.bass as bass
import concourse.tile as tile
from concourse import mybir
from concourse.bass_utils import run_bass_kernel_spmd

F32 = mybir.dt.float32
F32R = mybir.dt.float32r
BF16 = mybir.dt.bfloat16
AF = mybir.ActivationFunctionType
ALU = mybir.AluOpType

B, K, R, D = 8, 16, 100, 256
N = K * R            # 1600
H = W = 64
G = H * W            # 4096
NCORES = 8
BN_EPS = 1e-5
NTILES_A = [(0, 512), (512, 512)]          # psum chunk A: n-cols 0..1023
NTILES_B = [(1024, 512), (1536, 64)]       # psum chunk B: n-cols 1024..1599
NB = 576                                   # chunk B width


def _r(ap):
    return ap.bitcast(F32R)


def _host_constants():
    t = np.linspace(0.0, 1.0, R).astype(np.float64)
    basisT = np.stack(
        [comb(3, c) * t**c * (1.0 - t) ** (3 - c) for c in range(4)], axis=0
    ).astype(np.float32)                               # [4, 100]

    onehot = np.zeros((K, N), np.float32)
    for k in range(K):
        onehot[k, k * R : (k + 1) * R] = 1.0           # [16, 1600]

    xs = np.linspace(0.0, 1.0, W).astype(np.float64)
    gx = np.tile(xs, H)                                 # g % 64
    gy = np.repeat(xs, W)                               # g // 64
    grid5 = np.stack(
        [2.0 * gx, 2.0 * gy, gx**2 + gy**2, np.ones(G), -np.ones(G)], axis=0
    ).astype(np.float32)                                # [5, 4096]
    # replicated at partition offsets 0/32/64/96 for 16-way PE tile packing
    gridT = np.zeros((128, G), np.float32)
    for i in range(4):
        gridT[32 * i : 32 * i + 5, :] = grid5
    # sign mask for building rhs rows, replicated at the same offsets
    mask101 = np.zeros((16, 101), np.float32)
    pat = np.array([1.0, 1.0, -1.0, -1.0, 0.0], np.float32)
    for i in range(4):
        mask101[:, 32 * i : 32 * i + 5] = pat[None, :]
    id128 = np.eye(128, dtype=np.float32)
    return basisT, onehot, gridT, mask101, id128


def _split_multi_waits(nc):
    """Walrus codegen in this toolchain accepts one sync-wait per instruction;
    carry extra waits on same-engine NoOps inserted just before."""
    for f in nc.m.functions:
        for blk in f.blocks:
            idx = 0
            while idx < len(blk.instructions):
                inst = blk.instructions[idx]
                si = inst.sync_info
                if si is not None and len(si.on_wait) > 1:
                    waits = list(si.on_wait)
                    for j, w in enumerate(waits[:-1]):
                        nop = mybir.InstNoOp(name=f"WSPLIT-{nc.next_id()}",
                                             ins=[], outs=[])
                        nop.engine = inst.engine
                        nop.sync_info = mybir.SyncInfo(on_wait=[w], on_update=[])
                        blk.instructions.insert(idx + j, nop)
                    idx += len(waits) - 1
                    inst.sync_info = mybir.SyncInfo(on_wait=[waits[-1]],
                                                    on_update=list(si.on_update))
                idx += 1


def _build_program():
    nc = bass.Bass()

    # ---- DRAM I/O declarations (shapes only; data supplied per core) ----
    dr = {}

    def din(name, shape, dt=F32):
        dr[name] = nc.dram_tensor(name, list(shape), dt, kind="ExternalInput")
        return dr[name]

    din("cpT", (2, 64))            # encoder input, cols n=(k,cp)
    din("cpd", (4, 32))            # bezier lhsT, cols 2k+d
    din("basisT", (4, 100))
    din("onehot", (16, N), F32R)
    din("gridT", (128, G), BF16)
    din("mask101", (16, 101))
    din("id128", (128, 128))
    din("encw1", (2, 64)), din("encb1", (64, 1))
    din("encw2", (64, 128)), din("encb2", (128, 1))
    din("encw3", (128, 256)), din("encb3", (128, 2))
    din("aggw1", (128, 2, 2, 128)), din("aggb1", (128, 2))
    din("aggw2", (128, 2, 2, 128)), din("aggb2", (128, 2))
    din("kerw1", (128, 2, 3, 64), F32R), din("kerb1", (64, 3))
    din("kerw2", (64, 32)), din("kerb2", (32, 1))
    din("kerw3", (32, 1))
    din("aw1", (128, 2, 3, 256), F32R), din("ab1row", (1, 3, 256), F32R)
    din("w1p", (2, 256), F32R)
    din("attw2", (128, 2, 128), BF16), din("attb2", (128, 1))
    din("attw3", (128, 1), F32R)
    din("w1im", (128, 3, 16), F32R), din("fusb1", (16, 1))
    din("w2im", (128, 3, 8), F32R), din("fusb2", (8, 1))
    din("w3sel", (8, 8, 8), F32R)
    out_dram = nc.dram_tensor("out", [1, G], F32, kind="ExternalOutput")

    # imm scalars get baked at build time from the actual inputs:
    # we return a closure that finishes the build given those values.
    def finish(attb3, kerb3, fusb3, bn3f, bn3b):
        with ExitStack() as ctx:
            tc = ctx.enter_context(tile.TileContext(nc))
            cpool = ctx.enter_context(tc.tile_pool(name="consts", bufs=1))
            wpool = ctx.enter_context(tc.tile_pool(name="work", bufs=1))

            # ---- load constants/weights to SBUF (big late-use ones last) ----
            sb = {}
            for name, shape in [
                ("cpT", (2, 64)), ("cpd", (4, 32)), ("basisT", (4, 100)),
                ("mask101", (16, 101)),
                ("encw1", (2, 64)), ("encb1", (64, 1)),
                ("encw2", (64, 128)), ("encb2", (128, 1)),
                ("encw3", (128, 256)), ("encb3", (128, 2)),
                ("aggw1", (128, 2, 2, 128)), ("aggb1", (128, 2)),
                ("aggw2", (128, 2, 2, 128)), ("aggb2", (128, 2)),
                ("kerw1", (128, 2, 3, 64)), ("kerb1", (64, 3)),
                ("kerw2", (64, 32)), ("kerb2", (32, 1)),
                ("kerw3", (32, 1)),
                ("aw1", (128, 2, 3, 256)), ("ab1row", (1, 3, 256)),
                ("attw2", (128, 2, 128)), ("attb2", (128, 1)), ("attw3", (128, 1)),
                ("w1im", (128, 3, 16)), ("fusb1", (16, 1)),
                ("w2im", (128, 3, 8)), ("fusb2", (8, 1)),
                ("w3sel", (8, 8, 8)),
                ("onehot", (16, N)), ("id128", (128, 128)),
                ("gridT", (128, G)),
            ]:
                sb[name] = cpool.tile(list(shape), dr[name].dtype,
                                      name=f"sb_{name}")
                nc.sync.dma_start(out=sb[name][...], in_=dr[name][...])

            ones2 = cpool.tile([2, 1], F32R)
            nc.vector.memset(ones2[...].bitcast(F32), 1.0)
            ones16 = cpool.tile([1, 16], F32R)
            nc.vector.memset(ones16[...].bitcast(F32), 1.0)
            kerb3_t = cpool.tile([1, 1], F32)
            nc.vector.memset(kerb3_t[...], float(kerb3))
            nattb3_t = cpool.tile([1, 1], F32)
            nc.vector.memset(nattb3_t[...], float(-attb3))
            fusb3_t = cpool.tile([8, 1], F32)
            nc.vector.memset(fusb3_t[...], float(fusb3))

            # conv buffers allocated early; border memsets overlap early phases
            cvsb = ctx.enter_context(tc.tile_pool(name="conv_sbuf", bufs=1))
            # disjoint lifetimes share a slot: mpad dies once imY is built
            # (c2u reuses it)
            mpad = cvsb.tile([3, 66, 66], F32R, tag="cshare1")
            mTs = cvsb.tile([16, 2, 3, 128], F32R)
            c1p = cvsb.tile([16, 66, 66], F32R)
            imY = cvsb.tile([128, 64, 66], F32R)
            imY2 = cvsb.tile([48, 64, 66], F32R)
            c2u = cvsb.tile([8, 64, 64], F32R, tag="cshare1")
            sg = cvsb.tile([8, 512], F32)
            for t in (mpad, c1p):
                nc.vector.memset(t[:, 0:1, :].bitcast(F32), 0.0)     # top row
                nc.vector.memset(t[:, 65:66, :].bitcast(F32), 0.0)   # bottom row
                nc.vector.memset(t[:, 1:65, 0:1].bitcast(F32), 0.0)  # left col
                nc.vector.memset(t[:, 1:65, 65:66].bitcast(F32), 0.0)  # right col

            # ============ Phase 1: encoder + agg (feature-major) ============
            h1 = wpool.tile([64, 64], F32)
            h2 = wpool.tile([128, 64], F32)
            h3 = wpool.tile([128, 2, 64], F32)
            m = wpool.tile([128, 2, 16], F32)
            g1 = wpool.tile([128, 2, 16], F32)
            S = wpool.tile([128, 2, 16], F32R)

            with tc.tile_pool(name="pp1", bufs=4, space="PSUM") as pp1:
                ps = pp1.tile([64, 64], F32, tag="pp1t")
                nc.tensor.matmul(ps[...], sb["encw1"][...], sb["cpT"][...],
                                 start=True, stop=True)
                nc.scalar.activation(h1[...], ps[...], AF.Relu, bias=sb["encb1"][:, 0:1])

                ps2 = pp1.tile([128, 64], F32, tag="pp1t")
                nc.tensor.matmul(ps2[...], sb["encw2"][...], h1[...],
                                 start=True, stop=True)
                nc.scalar.activation(h2[...], ps2[...], AF.Relu, bias=sb["encb2"][:, 0:1])

                for fh in range(2):
                    ps3 = pp1.tile([128, 64], F32, tag="pp1t")
                    nc.tensor.matmul(ps3[...], sb["encw3"][:, 128 * fh : 128 * (fh + 1)],
                                     h2[...], start=True, stop=True)
                    nc.scalar.activation(h3[:, fh, :], ps3[...], AF.Relu,
                                         bias=sb["encb3"][:, fh : fh + 1])

                # mean over 4 control points (the 0.25 is folded into aggw1)
                h3r = h3[...].rearrange("p h (k c) -> p h k c", c=4)
                nc.vector.tensor_add(m[...], h3r[:, :, :, 0], h3r[:, :, :, 1])
                nc.vector.tensor_add(m[...], m[...], h3r[:, :, :, 2])
                nc.vector.tensor_add(m[...], m[...], h3r[:, :, :, 3])

                for dst, wname, bname, rhs in ((g1, "aggw1", "aggb1", m),
                                               (S, "aggw2", "aggb2", g1)):
                    for fh in range(2):
                        psg = pp1.tile([128, 16], F32, tag="pp1t")
                        for inh in range(2):
                            nc.tensor.matmul(psg[...], sb[wname][:, inh, fh, :],
                                             rhs[:, inh, :],
                                             start=(inh == 0), stop=(inh == 1))
                        nc.scalar.activation(dst[:, fh, :], psg[...], AF.Relu,
                                             bias=sb[bname][:, fh : fh + 1])

                # ============ Phase 2: Bezier points ============
                P = wpool.tile([2, N], F32)
                pmn2 = wpool.tile([2, 2], F32)
                pmx2 = wpool.tile([2, 2], F32)
                for hf in range(2):
                    psb = pp1.tile([2, 800], F32, tag="pp1b", bufs=2)
                    for k8 in range(8):
                        k = 8 * hf + k8
                        nc.tensor.matmul(psb[:, 100 * k8 : 100 * k8 + 100],
                                         sb["cpd"][:, 2 * k : 2 * k + 2],
                                         sb["basisT"][...], start=True, stop=True)
                    nc.vector.tensor_copy(P[:, 800 * hf : 800 * hf + 800],
                                          psb[...])
                    nc.vector.tensor_reduce(pmn2[:, hf : hf + 1], psb[...],
                                            axis=mybir.AxisListType.X, op=ALU.min)
                    nc.vector.tensor_reduce(pmx2[:, hf : hf + 1], psb[...],
                                            axis=mybir.AxisListType.X, op=ALU.max)

                pmin = wpool.tile([2, 1], F32)
                pmax = wpool.tile([2, 1], F32)
                rec = wpool.tile([2, 1], F32)
                nc.vector.tensor_reduce(pmin[...], pmn2[...], axis=mybir.AxisListType.X,
                                        op=ALU.min)
                nc.vector.tensor_reduce(pmax[...], pmx2[...], axis=mybir.AxisListType.X,
                                        op=ALU.max)
                nc.vector.tensor_tensor(rec[...], pmax[...], pmin[...], op=ALU.subtract)
                nc.vector.tensor_scalar_add(rec[...], rec[...], 1e-8)
                nc.vector.reciprocal(rec[...], rec[...])
                # pn = (P - pmin) * rec, in place
                nc.vector.tensor_scalar(P[...], P[...], pmin[...], rec[...],
                                        op0=ALU.subtract, op1=ALU.mult)

                P2 = wpool.tile([2, N], F32R)
                nc.vector.tensor_mul(P2[...], P[...], P[...])
                sqrow = wpool.tile([1, N], F32)
                for t0, w in NTILES_A + NTILES_B:
                    pss = pp1.tile([1, 512], F32, tag="pp1t")
                    nc.tensor.matmul(pss[:, :w], ones2[...], P2[:, t0 : t0 + w],
                                     start=True, stop=True)
                    nc.vector.tensor_copy(sqrow[:, t0 : t0 + w], pss[:, :w])

                # ---- ker MLP for all 3 scales -> cT [16, 3] ----
                e48 = wpool.tile([1, 48], F32)
                r48 = wpool.tile([1, 48], F32)
                cT = wpool.tile([16, 3], F32)
                k1 = wpool.tile([64, 16], F32)
                k2 = wpool.tile([32, 16], F32)
                for s in range(3):
                    psk1 = pp1.tile([64, 16], F32, tag="pp1t")
                    for inh in range(2):
                        nc.tensor.matmul(psk1[...], sb["kerw1"][:, inh, s, :],
                                         S[:, inh, :], start=(inh == 0), stop=(inh == 1))
                    nc.scalar.activation(k1[...], psk1[...], AF.Relu,
                                         bias=sb["kerb1"][:, s : s + 1])
                    psk2 = pp1.tile([32, 16], F32, tag="pp1t")
                    nc.tensor.matmul(psk2[...], sb["kerw2"][...], k1[...],
                                     start=True, stop=True)
                    nc.scalar.activation(k2[...], psk2[...], AF.Relu,
                                         bias=sb["kerb2"][:, 0:1])
                    psk3 = pp1.tile([1, 16], F32, tag="pp1t")
                    nc.tensor.matmul(psk3[...], sb["kerw3"][...], k2[...],
                                     start=True, stop=True)
                    # e48 k-major (col 3k+s) = exp(z + kerb3)
                    e48v = e48[...].rearrange("p (k s) -> p k s", s=3)
                    nc.scalar.activation(e48v[:, :, s], psk3[...],
                                         AF.Exp, bias=kerb3_t[...])

                # softplus, then c = 1/(2*sp^2) = (recip(sp)/sqrt(2))^2
                nc.vector.tensor_scalar_add(e48[...], e48[...], 1.0)
                nc.scalar.activation(e48[...], e48[...], AF.Ln)
                nc.vector.reciprocal(r48[...], e48[...])
                nc.scalar.activation(e48[...], r48[...], AF.Square,
                                     scale=0.7071067811865476)
                # transpose [1,48] -> [16k, 3s]
                nc.sync.dma_start(out=cT[...], in_=e48[...])

            # assembled point-side tensors
            xaug = wpool.tile([18, N], F32R)
            nc.sync.dma_start(out=xaug[0:16, :], in_=sb["onehot"][...])
            nc.sync.dma_start(out=xaug[16:18, :], in_=_r(P[...]))
            prep101 = wpool.tile([101, N], F32)
            nc.vector.memset(prep101[...], 1.0)
            for i in range(4):
                nc.sync.dma_start(out=prep101[32 * i : 32 * i + 2, :], in_=P[...])
                nc.sync.dma_start(out=prep101[32 * i + 3 : 32 * i + 4, :],
                                  in_=sqrow[...])

            # w1aug rows 16,17 = w1p for every scale (broadcast DMA)
            w1aug = wpool.tile([18, 3, 256], F32R)
            for s in range(3):
                nc.sync.dma_start(out=w1aug[16:18, s, :], in_=dr["w1p"][...])

            # ====== Phases 3+4 interleaved: per-scale att MLP + KDE ======
            rhs5 = [
                wpool.tile([101, N], BF16, name=f"rhs5_{s}", tag=f"rhs5_{s}")
                for s in range(3)
            ]
            m_all = wpool.tile([128, 3, 32], F32)

            with (
                tc.tile_pool(name="scale_work", bufs=1) as spool,
                tc.tile_pool(name="mlp_ps", bufs=4, space="PSUM") as pp3,
            ):
                # the 3 scales' chains are independent: emit stage-major so
                # the PE streams while DVE/ACT chase, no serial per-scale wall
                a1 = [spool.tile([128, 2, N], BF16, name=f"a1_{s}", tag=f"a1_{s}")
                      for s in range(3)]
                a2 = [spool.tile([128, N], F32R, name=f"a2_{s}", tag=f"a2_{s}")
                      for s in range(3)]
                esp = [spool.tile([1, N], BF16, name=f"esp_{s}", tag=f"esp_{s}")
                       for s in range(3)]
                cneg5 = [spool.tile([16, 101], F32R, name=f"cneg5_{s}",
                         tag=f"cneg5_{s}") for s in range(3)]
                for s in range(3):
                    pscf = pp3.tile([16, 256], F32, tag="mlp")
                    nc.tensor.matmul(pscf[...], S[:, 0, :], sb["aw1"][:, 0, s, :],
                                     start=True, stop=False)
                    nc.tensor.matmul(pscf[...], S[:, 1, :], sb["aw1"][:, 1, s, :],
                                     start=False, stop=False)
                    nc.tensor.matmul(pscf[...], ones16[...],
                                     sb["ab1row"][:, s, :], start=False, stop=True)
                    nc.vector.tensor_copy(_r(w1aug[0:16, s, :]), pscf[...])
                    nc.vector.tensor_scalar_mul(cneg5[s][...], _r(sb["mask101"][...]),
                                                cT[:, s : s + 1])
                for fh in range(2):
                    for t0, w in NTILES_A + NTILES_B:
                        for s in range(3):
                            psa = pp3.tile([128, 512], F32, tag="mlp")
                            nc.tensor.matmul(psa[:, :w],
                                             w1aug[:, s, 128 * fh : 128 * (fh + 1)],
                                             xaug[:, t0 : t0 + w],
                                             start=True, stop=True)
                            nc.vector.tensor_scalar_max(a1[s][:, fh, t0 : t0 + w],
                                                        psa[:, :w], 0.0)
                for t0, w in NTILES_A + NTILES_B:
                    for s in range(3):
                        psa2 = pp3.tile([128, 512], F32, tag="mlp")
                        for fh in range(2):
                            nc.tensor.matmul(psa2[:, :w], sb["attw2"][:, fh, :],
                                             a1[s][:, fh, t0 : t0 + w],
                                             start=(fh == 0), stop=(fh == 1))
                        nc.vector.tensor_scalar(a2[s][:, t0 : t0 + w], psa2[:, :w],
                                                sb["attb2"][:, 0:1], 0.0,
                                                op0=ALU.add, op1=ALU.max)
                for t0, w in NTILES_A + NTILES_B:
                    for s in range(3):
                        psz = pp3.tile([1, 512], F32, tag="mlp")
                        nc.tensor.matmul(psz[:, :w], sb["attw3"][...],
                                         a2[s][:, t0 : t0 + w], start=True, stop=True)
                        # exp(-(z + attb3))
                        nc.scalar.activation(esp[s][:, t0 : t0 + w], psz[:, :w],
                                             AF.Exp, bias=nattb3_t[...], scale=-1.0)
                for s in range(3):
                    nc.vector.tensor_scalar_add(esp[s][...], esp[s][...], 1.0)
                    nc.scalar.activation(esp[s][...], esp[s][...], AF.Ln)
                for t0, w in NTILES_A + NTILES_B:
                    for s in range(3):
                        psc = pp3.tile([101, 512], F32, tag="mlp")
                        nc.tensor.matmul(psc[:, :w], cneg5[s][...],
                                         sb["onehot"][:, t0 : t0 + w],
                                         start=True, stop=True)
                        nc.vector.tensor_mul(rhs5[s][:, t0 : t0 + w], psc[:, :w],
                                             prep101[:, t0 : t0 + w])
                # rows 32i+4 <- softplus(-z-b3)  (partition move via DMA)
                for s in range(3):
                    for i in range(4):
                        nc.sync.dma_start(out=rhs5[s][32 * i + 4 : 32 * i + 5, :],
                                          in_=esp[s][...])

            # ====== Phase 4: KDE (single exp per block, ACT accumulator) ====
            with (
                tc.tile_pool(name="kde_ps", bufs=2, space="PSUM") as kpp,
                tc.tile_pool(name="kde_scratch", bufs=2) as ksp,
            ):
                for s in range(3):
                    for gb in range(32):
                        kps = kpp.tile([128, 2048], F32, tag="kps")
                        for ti, (t0, w) in enumerate(NTILES_A + NTILES_B):
                            for j in range(4):
                                nc.tensor.matmul(
                                    kps[32 * j : 32 * (j + 1),
                                        512 * ti : 512 * ti + w],
                                    sb["gridT"][32 * ti : 32 * ti + 5,
                                               128 * gb + 32 * j : 128 * gb + 32 * (j + 1)],
                                    rhs5[s][32 * ti : 32 * ti + 5, t0 : t0 + w],
                                    start=True, stop=True,
                                    tile_position=(32 * ti, 32 * j),
                                )
                        scr = ksp.tile([128, N], BF16, tag="scr")
                        nc.scalar.activation(scr[...], kps[:, 0:N], AF.Exp,
                                             accum_out=m_all[:, s, gb : gb + 1])

                        # after each half of the grid, transpose the
                        # finished 16 gb-columns and scatter them so only the
                        # final half of scale 2 sits on the critical path.
                        if gb in (15, 31):
                            g0 = gb - 15
                            pst = kpp.tile([16, 128], F32, tag="kps", bufs=2)
                            nc.tensor.transpose(pst[...],
                                                m_all[:, s, g0 : g0 + 16],
                                                sb["id128"][...])
                            hb = g0 // 16
                            nc.vector.tensor_copy(mTs[:, hb, s, :], pst[...])
                            nc.gpsimd.dma_start(
                                out=mpad[s : s + 1, 1 + 2 * g0 : 33 + 2 * g0,
                                         1:65].rearrange(
                                    "a (gb ph) x -> a gb ph x", ph=2),
                                in_=mTs[:, hb, s, :],
                            )
                            # im2col rows for this channel:
                            # imY[3dy+s, y, x] = mpad[s, y+dy, x]
                            y0, y1 = (0, 30) if g0 == 0 else (30, 64)
                            for dy in range(3):
                                nc.gpsimd.dma_start(
                                    out=imY[3 * dy + s : 3 * dy + s + 1,
                                            y0:y1, :],
                                    in_=mpad[s : s + 1, y0 + dy : y1 + dy, :])

            # ============ Phase 5: conv head ============
            with tc.tile_pool(name="conv_ps", bufs=4, space="PSUM") as cvp:
                for st in range(8):
                    ps1 = cvp.tile([16, 512], F32, tag="cv1")
                    for dx in range(3):
                        nc.tensor.matmul(
                            ps1[...], sb["w1im"][0:9, dx, :],
                            imY[0:9, st * 8 : st * 8 + 8, dx : dx + 64],
                            start=(dx == 0), stop=(dx == 2),
                        )
                    nc.vector.tensor_scalar(c1p[:, 1 + st * 8 : 9 + st * 8, 1:65],
                                            ps1[...], sb["fusb1"][:, 0:1], 0.0,
                                            op0=ALU.add, op1=ALU.max)
                # imY2[16dy+c, y, x] = c1p[c, y+dy, x]; per-strip chunks
                # (issued on the idle ACT queue) so conv2 strips can start
                # while conv1 is still running
                for st in range(8):
                    for dy in range(3):
                        nc.scalar.dma_start(
                            out=imY2[16 * dy : 16 * dy + 16,
                                     st * 8 : st * 8 + 8, :],
                            in_=c1p[:, st * 8 + dy : st * 8 + 8 + dy, :])
                for st in range(8):
                    ps2c = cvp.tile([8, 512], F32, tag="cv2")
                    for dx in range(3):
                        nc.tensor.matmul(
                            ps2c[...], sb["w2im"][0:48, dx, :],
                            imY2[:, st * 8 : st * 8 + 8, dx : dx + 64],
                            start=(dx == 0), stop=(dx == 2),
                        )
                    nc.vector.tensor_scalar(c2u[:, st * 8 : 8 + st * 8, :],
                                            ps2c[...], sb["fusb2"][:, 0:1], 0.0,
                                            op0=ALU.add, op1=ALU.max)
            with tc.tile_pool(name="conv3_ps", bufs=1, space="PSUM") as cvp3:
                # 1x1 conv with strip-selector weights: psum row st = w3 . c2u strip st
                ps3c = cvp3.tile([8, 512], F32, tag="cv3")
                for st in range(8):
                    nc.tensor.matmul(ps3c[...],
                                     sb["w3sel"][:, st, :],
                                     c2u[:, st * 8 : st * 8 + 8, :],
                                     start=(st == 0), stop=(st == 7))
                nc.scalar.activation(sg[...], ps3c[...], AF.Sigmoid,
                                     bias=fusb3_t[...])

            # bn3 affine, then store
            nc.vector.tensor_scalar(sg[...], sg[...], bn3f, bn3b,
                                    op0=ALU.mult, op1=ALU.add)
            nc.sync.dma_start(
                out=out_dram[...].rearrange("a (r x) -> (a r) x", r=8),
                in_=sg[...])

        _split_multi_waits(nc)
        return nc

    return nc, finish


def _prepare_maps(inputs):
    """Host-side weight folding; returns per-core in_maps (list of dicts)."""
    f = {k: np.asarray(v, dtype=np.float32) for k, v in inputs.items()}
    basisT, onehot, gridT, mask101, id128 = _host_constants()

    bn1f = f["bn1_g"] / np.sqrt(np.float32(1.0 + BN_EPS))
    bn2f = f["bn2_g"] / np.sqrt(np.float32(1.0 + BN_EPS))
    A = (bn1f * bn2f).astype(np.float32)                     # [256]
    C = (f["bn1_b"] * bn2f + f["bn2_b"]).astype(np.float32)  # [256]

    scales = (0.5, 1.0, 2.0)
    kerw1 = np.stack(
        [(s * A)[:, None] * f["ker_w1"] for s in scales], 0
    )  # [3,256,64]
    kerb1 = np.stack(
        [s * (C @ f["ker_w1"]) + f["ker_b1"] for s in scales], 1
    )  # [64,3]
    aw1f = np.stack(
        [(s * A)[:, None] * f["att_w1"][:D] for s in scales], 0
    )  # [3,256,256]
    ab1row = np.stack(
        [s * (C @ f["att_w1"][:D]) + f["att_b1"] for s in scales], 0
    ).reshape(1, 3, 256)

    w1im9 = f["fus_w1"].transpose(2, 1, 3, 0).reshape(9, 3, 16)   # [3dy+c, dx, o]
    w1im = np.zeros((128, 3, 16), np.float32)
    for q in range(4):
        w1im[32 * q : 32 * q + 9] = w1im9                          # 4 row-group copies
    w2im48 = f["fus_w2"].transpose(2, 1, 3, 0).reshape(48, 3, 8)   # [16dy+c, dx, o]
    w2im = np.zeros((128, 3, 8), np.float32)
    for q in range(2):
        w2im[64 * q : 64 * q + 48] = w2im48                        # 2 row-group copies
    w3 = f["fus_w3"].reshape(8)
    w3sel = np.zeros((8, 8, 8), np.float32)                       # [c, st, r]
    for st in range(8):
        w3sel[:, st, st] = w3

    import ml_dtypes
    common = {
        "basisT": basisT,
        "onehot": onehot,
        "gridT": gridT.astype(ml_dtypes.bfloat16),
        "mask101": mask101,
        "id128": id128,
        "encw1": f["enc_w1"],
        "encb1": f["enc_b1"].reshape(64, 1),
        "encw2": f["enc_w2"],
        "encb2": f["enc_b2"].reshape(128, 1),
        "encw3": f["enc_w3"],
        "encb3": f["enc_b3"].reshape(2, 128).T.copy(),
        "aggw1": (0.25 * f["agg_w1"]).reshape(2, 128, 2, 128).transpose(1, 0, 2, 3).copy(),
        "aggb1": f["agg_b1"].reshape(2, 128).T.copy(),
        "aggw2": f["agg_w2"].reshape(2, 128, 2, 128).transpose(1, 0, 2, 3).copy(),
        "aggb2": f["agg_b2"].reshape(2, 128).T.copy(),
        "kerw1": kerw1.reshape(3, 2, 128, 64).transpose(2, 1, 0, 3).copy(),
        "kerb1": kerb1,
        "kerw2": f["ker_w2"],
        "kerb2": f["ker_b2"].reshape(32, 1),
        "kerw3": f["ker_w3"],
        "aw1": aw1f.reshape(3, 2, 128, 256).transpose(2, 1, 0, 3).copy(),
        "ab1row": ab1row,
        "w1p": f["att_w1"][D : D + 2].copy(),
        "attw2": f["att_w2"].reshape(2, 128, 128).transpose(1, 0, 2).astype(ml_dtypes.bfloat16),
        "attb2": f["att_b2"].reshape(128, 1),
        "attw3": f["att_w3"],
        "w1im": w1im,
        "fusb1": f["fus_b1"].reshape(16, 1),
        "w2im": w2im,
        "fusb2": f["fus_b2"].reshape(8, 1),
        "w3sel": w3sel,
    }
    common = {
        k: np.ascontiguousarray(v) if v.dtype == ml_dtypes.bfloat16
        else np.ascontiguousarray(v, dtype=np.float32)
        for k, v in common.items()
    }

    in_maps = []
    cp = f["control_points"]  # [8, 16, 4, 2]
    for c in range(NCORES):
        m = dict(common)
        m["cpT"] = np.ascontiguousarray(cp[c].reshape(64, 2).T)       # [2, 64]
        m["cpd"] = np.ascontiguousarray(cp[c].transpose(1, 0, 2).reshape(4, 32))
        in_maps.append(m)

    imm = dict(
        attb3=float(f["att_b3"][0]),
        kerb3=float(f["ker_b3"][0]),
        fusb3=float(f["fus_b3"][0]),
        bn3f=float(f["bn3_g"][0] / np.sqrt(1.0 + BN_EPS)),
        bn3b=float(f["bn3_b"][0]),
    )
    return in_maps, imm


def kernel(**inputs) -> np.ndarray:
    in_maps, imm = _prepare_maps(inputs)
    nc, finish = _build_program()
    nc = finish(**imm)
    res = run_bass_kernel_spmd(nc, in_maps, core_ids=list(range(NCORES)))
    kernel._last_results = res
    out = np.stack([r["out"].reshape(1, H, W) for r in res.results], axis=0)
    return out.astype(np.float32)
